# revision 1
# baseline (speedup 1.0000x reference)
"""Trainium2 Bass kernel for the PrimedGKA layer (gated linear attention with
Chebyshev query refinement), tensor-parallel over the 16 query heads across
8 NeuronCores (2 q-heads + their shared kv-head per core), out-projection
computed per-core against the core's Wo row-block; partial outputs summed on
the host (unshard of the sum-sharded output).

Precision plan: q/k/v datapath and all large matmuls in fp16 (PE runs 2-byte
matmuls at 1 cyc/row vs 4 for fp32) with fp32 PSUM accumulation; the decay
path (gate logits, cumulative log-decay G, the exp(G_t - G_s) mask build) and
the recurrent states stay fp32.

Self-contained: hardcodes all shapes from the problem spec.
"""
import numpy as np

B, T, D = 1, 1024, 1024
HQ, HKV, HK, HV = 16, 4, 64, 64
KW = 4
NCORES = 8
L = 128                 # chunk length
NCH = T // L            # 8 chunks
CHEB_DAMP = 0.25
EPS = 1e-6
QSCALE = HK ** -0.5

_PROG_CACHE = {}


def _build_program(dbg=False, reps=1):
    import concourse.bacc as bacc
    import concourse.mybir as mybir
    from concourse.tile import TileContext

    dt = mybir.dt
    f32 = dt.float32
    f16 = dt.float16
    AF = mybir.ActivationFunctionType
    ALU = mybir.AluOpType
    X = mybir.AxisListType.X

    nc = bacc.Bacc("TRN2", target_bir_lowering=False, debug=False,
                   num_devices=NCORES)

    xT16 = nc.dram_tensor("xT16", [D, T], f16, kind="ExternalInput")
    wcat = nc.dram_tensor("wcat", [D, 256], f16, kind="ExternalInput")
    wcv = nc.dram_tensor("wcv", [256, KW], f32, kind="ExternalInput")
    wg = nc.dram_tensor("wg", [D, 5], f16, kind="ExternalInput")
    wo = nc.dram_tensor("wo", [128, D], f16, kind="ExternalInput")
    alog = nc.dram_tensor("alog", [1, 2], f32, kind="ExternalInput")
    dtb5 = nc.dram_tensor("dtb5", [1, 5], f16, kind="ExternalInput")
    iden = nc.dram_tensor("iden", [128, 128], f32, kind="ExternalInput")
    iden16 = nc.dram_tensor("iden16", [128, 128], f16, kind="ExternalInput")
    umask = nc.dram_tensor("umask", [128, 128], f32, kind="ExternalInput")
    nmask = nc.dram_tensor("nmask", [128, 128], f32, kind="ExternalInput")
    outp = nc.dram_tensor("outp", [T, D], f32, kind="ExternalOutput")
    if dbg:
        dqkv = nc.dram_tensor("dqkv", [2, 128, T], f16, kind="ExternalOutput")
        dgate = nc.dram_tensor("dgate", [NCH, 128, 5], f32, kind="ExternalOutput")
        dkvtm = nc.dram_tensor("dkvtm", [NCH, 128, 128], f16, kind="ExternalOutput")
        dgt = nc.dram_tensor("dgt", [NCH, 128, 256], f16, kind="ExternalOutput")
        dhm = nc.dram_tensor("dhm", [NCH, 2, 64, 128], f32, kind="ExternalOutput")
        don = nc.dram_tensor("don", [NCH, 128, 128], f16, kind="ExternalOutput")

    with TileContext(nc) as tc:
      import contextlib
      for _rep in range(reps):
        ctx = contextlib.ExitStack()
        with ctx:
            pers = ctx.enter_context(tc.tile_pool(name="pers", bufs=1))
            p_gl = ctx.enter_context(tc.tile_pool(name="p_gl", bufs=8))
            p_big = ctx.enter_context(tc.tile_pool(name="p_big", bufs=9))
            p_gt = ctx.enter_context(tc.tile_pool(name="p_gt", bufs=9))
            p_gb = ctx.enter_context(tc.tile_pool(name="p_gb", bufs=9))
            p_sm = ctx.enter_context(tc.tile_pool(name="p_sm", bufs=9))
            p_kv = ctx.enter_context(tc.tile_pool(name="p_kv", bufs=9))
            p_hm = ctx.enter_context(tc.tile_pool(name="p_hm", bufs=10))
            p_xq = ctx.enter_context(tc.tile_pool(name="p_xq", bufs=10))
            p_out = ctx.enter_context(tc.tile_pool(name="p_out", bufs=6))
            ps_all = ctx.enter_context(tc.tile_pool(name="ps_all", bufs=8, space="PSUM"))
            ps_pj = ps_big = ps_med = ps_sm = ps_all

            # ---- persistent loads (xt split per d-tile so compute starts early) ----
            wcat_sb = pers.tile([128, 8, 256], f16)
            nc.sync.dma_start(out=wcat_sb[:], in_=wcat[:].rearrange("(a p) c -> p a c", p=128))
            wcv_sb = pers.tile([128, 2, KW], f32)
            nc.sync.dma_start(out=wcv_sb[:], in_=wcv[:].rearrange("(a p) k -> p a k", p=128))
            wg_sb = pers.tile([128, 8, 5], f16)
            nc.sync.dma_start(out=wg_sb[:], in_=wg[:].rearrange("(a p) c -> p a c", p=128))
            xt16_sb = pers.tile([128, 8, T], f16)
            for d in range(8):
                nc.sync.dma_start(out=xt16_sb[:, d, :], in_=xT16[d * 128:(d + 1) * 128, :])
            wo_sb = pers.tile([128, D], f16)
            nc.sync.dma_start(out=wo_sb[:], in_=wo[:])
            alog_sb = pers.tile([1, 2], f32)
            nc.sync.dma_start(out=alog_sb[:], in_=alog[:])
            dtb5_sb = pers.tile([1, 5], f16)
            nc.sync.dma_start(out=dtb5_sb[:], in_=dtb5[:])
            iden_sb = pers.tile([128, 128], f32)
            nc.sync.dma_start(out=iden_sb[:], in_=iden[:])
            iden16_sb = pers.tile([128, 128], f16)
            nc.sync.dma_start(out=iden16_sb[:], in_=iden16[:])
            um_sb = pers.tile([128, 128], f32)
            nc.sync.dma_start(out=um_sb[:], in_=umask[:])
            nm_sb = pers.tile([128, 128], f32)
            nc.sync.dma_start(out=nm_sb[:], in_=nmask[:])

            ones128 = pers.tile([1, 128], f32)
            nc.vector.memset(ones128[:], 1.0)
            ones128h = pers.tile([1, 128], f16)
            nc.vector.memset(ones128h[:], 1.0)
            zeros_hm = pers.tile([64, 128], f32)
            nc.vector.memset(zeros_hm[:], 0.0)
            zeros_hm16 = pers.tile([64, 128], f16)
            nc.vector.memset(zeros_hm16[:], 0.0)
            # q is used UNSCALED (no HK^-0.5): the output is linear in q and
            # the per-head rmsnorm absorbs a global scale exactly, provided the
            # rms eps is scaled by (1/QSCALE)^2 = HK.
            epsb = pers.tile([128, 1], f32)
            nc.vector.memset(epsb[:], EPS * HK)

            Rq = pers.tile([128, T], f16)     # raw q projection (pre-conv), fm
            Rkv = pers.tile([128, T], f16)    # raw k|v projection, fm
            Cq = pers.tile([128, T], f16)
            Ckv = pers.tile([128, T], f16)
            Sq = pers.tile([128, T], f16)     # silu(conv(q)), fm
            Skv = pers.tile([128, T], f16)    # silu(conv(k|v)), fm
            qst = pers.tile([64, 2 * T], f16)  # q heads stacked per chunk, scaled

            # ---- q/k/v projections (fp16): qkvT[c, t] = sum_d W[d, c] xT[d, t] ----
            for ct, dst in ((1, Rkv), (0, Rq)):
                c0 = ct * 128
                for th in range(2):
                    ps = ps_pj.tile([128, 512], f32, tag="ps")
                    for d in range(8):
                        nc.tensor.matmul(
                            ps[:],
                            wcat_sb[:, d, c0:c0 + 128],
                            xt16_sb[:, d, th * 512:(th + 1) * 512],
                            start=(d == 0), stop=(d == 7),
                        )
                    nc.vector.tensor_copy(dst[:, th * 512:(th + 1) * 512], ps[:])

            # ---- causal depthwise conv + silu; kv tile on DVE (critical for the
            # chunk preludes), q tile offloaded to the otherwise-idle GpSimd
            # (which lacks scalar_tensor_tensor, so tap-mult and accumulate are
            # separate TensorTensor ops with a free-dim-broadcast weight) ----
            w = lambda ct, k: wcv_sb[:, ct, k:k + 1]
            for (R, C, S, ct) in ((Rkv, Ckv, Skv, 1), (Rq, Cq, Sq, 0)):
                nc.vector.tensor_scalar(C[:, 0:T], R[:, 0:T], w(ct, 3), None, ALU.mult)
                for tap, sh in ((2, 1), (1, 2), (0, 3)):
                    nc.vector.scalar_tensor_tensor(
                        C[:, sh:T], R[:, 0:T - sh], w(ct, tap), C[:, sh:T],
                        op0=ALU.mult, op1=ALU.add)
                nc.scalar.activation(S[:], C[:], AF.Silu)

            # ---- per-head gate constants broadcast over 128 partitions ----
            era = pers.tile([1, 2], f32)
            nc.scalar.activation(era[:], alog_sb[:], AF.Exp)
            ps_bc = ps_sm.tile([128, 128], f32, tag="ps")
            nc.tensor.matmul(ps_bc[:, 0:2], ones128[:], era[:], start=True, stop=True)
            negea_bc = pers.tile([128, 2], f32)
            nc.vector.tensor_scalar(negea_bc[:], ps_bc[:, 0:2], -1.0, None, ALU.mult)

            # ---- stacked, scaled q:  qst[f, ci*256 + h*128 + t] ----
            qv = qst[:].rearrange("p (c h t) -> p c h t", c=NCH, h=2)
            for h in range(2):
                nc.vector.tensor_copy(
                    qv[:, :, h, :],
                    Sq[h * 64:(h + 1) * 64, :].rearrange("p (c t) -> p c t", c=NCH))

            if dbg and _rep == 0:
                nc.sync.dma_start(out=dqkv[0], in_=Sq[:])
                nc.sync.dma_start(out=dqkv[1], in_=Skv[:])

            # ---- chunked recurrence: pass 1 computes every chunk's gates,
            # decay masks, normalized k/v and the (cheap, serial) state chain;
            # pass 2 then streams all chunks' operator applications ----
            hm_prev = [zeros_hm, zeros_hm]
            hm16_prev = [zeros_hm16, zeros_hm16]
            hm16_states = []   # per chunk: incoming fp16 [H|M] per head
            chunk_ctx = []
            for grp in (range(0, 4), range(4, 8)):
                grp = list(grp)
                sl = {ci: slice(ci * L, (ci + 1) * L) for ci in grp}
                d_ = {}
                def step(nm, ci, pool, shape, dt_, ptag=None):
                    t = pool.tile(shape, dt_, tag=ptag or nm, name=f"{nm}{ci}")
                    d_.setdefault(nm, {})[ci] = t
                    return t
                # gate projections (time-major, fp16 x, dt_bias folded in)
                for ci in grp:
                    g5 = step("g5", ci, ps_sm, [128, 128], f32, "ps")
                    for d in range(8):
                        nc.tensor.matmul(g5[:, 0:5], xt16_sb[:, d, sl[ci]], wg_sb[:, d, :],
                                         start=(d == 0), stop=False)
                    nc.tensor.matmul(g5[:, 0:5], ones128h[:], dtb5_sb[:],
                                     start=False, stop=True)
                    if dbg and _rep == 0:
                        gtm = step("gtm", ci, p_sm, [128, 5], f32)
                        nc.vector.tensor_copy(gtm[:], g5[:, 0:5])
                        nc.sync.dma_start(out=dgate[ci], in_=gtm[:])
                for ci in grp:
                    nc.scalar.activation(step("e_a", ci, p_sm, [128, 2], f32)[:],
                                         d_["g5"][ci][:, 0:2], AF.Exp)
                for ci in grp:
                    nc.scalar.activation(step("sp_tm", ci, p_sm, [128, 2], f32)[:],
                                         d_["e_a"][ci][:], AF.Ln, bias=1.0)
                for ci in grp:
                    gp = step("g_pad", ci, p_sm, [128, 33], f32)
                    nc.vector.tensor_tensor(gp[:, 0:1], d_["sp_tm"][ci][:, 0:1],
                                            negea_bc[:, 0:1], ALU.mult)
                    nc.vector.tensor_tensor(gp[:, 32:33], d_["sp_tm"][ci][:, 1:2],
                                            negea_bc[:, 1:2], ALU.mult)
                for ci in grp:
                    nc.scalar.activation(step("e_g", ci, p_sm, [128, 3], f32)[:],
                                         d_["g5"][ci][:, 2:5], AF.Exp, scale=-1.0)
                for ci in grp:
                    nc.vector.tensor_scalar(step("d_g", ci, p_sm, [128, 3], f32)[:],
                                            d_["e_g"][ci][:], 1.0, None, ALU.add)
                for ci in grp:
                    nc.vector.reciprocal(step("ab_tm", ci, p_sm, [128, 3], f32)[:],
                                         d_["d_g"][ci][:])
                for ci in grp:
                    psG = step("psG", ci, ps_sm, [128, 128], f32, "ps")
                    nc.tensor.matmul(psG[:, 0:33], um_sb[:], d_["g_pad"][ci][:],
                                     start=True, stop=True)
                    psGr = step("psGr", ci, ps_sm, [128, 128], f32, "ps")
                    nc.tensor.matmul(psGr[0:33, :], d_["g_pad"][ci][:], um_sb[:],
                                     start=True, stop=True)
                for ci in grp:
                    G_sb = step("G_sb", ci, p_sm, [128, 2], f32)
                    nc.vector.tensor_copy(G_sb[:, 0:1], d_["psG"][ci][:, 0:1])
                    nc.vector.tensor_copy(G_sb[:, 1:2], d_["psG"][ci][:, 32:33])
                    grow = step("grow", ci, p_gl, [1, 256], f32)
                    nc.vector.tensor_copy(grow[0:1, 0:L], d_["psGr"][ci][0:1, :])
                    nc.vector.tensor_copy(grow[0:1, L:2 * L], d_["psGr"][ci][32:33, :])
                for ci in grp:
                    nc.scalar.activation(step("gamrow", ci, p_gl, [1, 256], f16)[:],
                                         d_["grow"][ci][:], AF.Exp)
                for ci in grp:
                    psGB = step("psGB", ci, ps_big, [128, 256], f32, "ps")
                    nc.tensor.matmul(psGB[:], ones128[:], d_["grow"][ci][:],
                                     start=True, stop=True)
                for ci in grp:
                    dm2 = step("dm2", ci, p_big, [128, 256], f32)
                    for h in range(2):
                        nc.vector.scalar_tensor_tensor(
                            dm2[:, h * L:(h + 1) * L], d_["psGB"][ci][:, h * L:(h + 1) * L],
                            d_["G_sb"][ci][:, h:h + 1], nm_sb[:],
                            op0=ALU.subtract, op1=ALU.min)
                for ci in grp:
                    nc.scalar.activation(step("gt", ci, p_gt, [128, 256], f16)[:],
                                         d_["dm2"][ci][:], AF.Exp)
                for ci in grp:
                    nc.scalar.activation(step("wend", ci, p_sm, [128, 2], f32)[:],
                                         d_["dm2"][ci][:].rearrange("p (a t) -> p a t", a=2)[:, :, L - 1],
                                         AF.Exp)
                for ci in grp:
                    psgb = step("psgb", ci, ps_med, [64, 256], f32, "ps")
                    nc.tensor.matmul(psgb[:], ones128h[0:1, 0:64], d_["gamrow"][ci][:],
                                     start=True, stop=True)
                for ci in grp:
                    nc.vector.tensor_copy(step("gb", ci, p_gb, [64, 256], f16)[:],
                                          d_["psgb"][ci][:])
                    nc.vector.tensor_copy(
                        step("gbL", ci, p_gb, [64, 2], f32)[:],
                        d_["psgb"][ci][:].rearrange("p (a t) -> p a t", a=2)[:, :, L - 1])
                # k/v transposes, k-norm, v beta scale
                for ci in grp:
                    pskt = step("pskt", ci, ps_sm, [128, 128], f16, "ps")
                    nc.tensor.transpose(pskt[:, 0:64], Skv[0:64, sl[ci]], iden16_sb[0:64, 0:64])
                    psvt = step("psvt", ci, ps_sm, [128, 128], f16, "ps")
                    nc.tensor.transpose(psvt[:, 0:64], Skv[64:128, sl[ci]], iden16_sb[64:128, 64:128])
                for ci in grp:
                    nc.scalar.activation(step("sqk", ci, p_sm, [128, 64], f32)[:],
                                         d_["pskt"][ci][:, 0:64], AF.Square)
                for ci in grp:
                    nc.vector.tensor_reduce(step("ssk", ci, p_sm, [128, 1], f32)[:],
                                            d_["sqk"][ci][:], X, ALU.add)
                for ci in grp:
                    nc.scalar.activation(step("lnk", ci, p_sm, [128, 1], f32)[:],
                                         d_["ssk"][ci][:], AF.Ln)
                for ci in grp:
                    nc.scalar.activation(step("nrk", ci, p_sm, [128, 1], f32)[:],
                                         d_["lnk"][ci][:], AF.Exp, scale=0.5)
                for ci in grp:
                    nc.vector.tensor_scalar(step("nre", ci, p_sm, [128, 1], f32)[:],
                                            d_["nrk"][ci][:], EPS, None, ALU.add)
                for ci in grp:
                    nc.vector.reciprocal(step("invk", ci, p_sm, [128, 1], f32)[:],
                                         d_["nre"][ci][:])
                for ci in grp:
                    kv_tm = step("kvtm", ci, p_kv, [128, 128], f16)
                    nc.vector.tensor_scalar(kv_tm[:, 0:64], d_["pskt"][ci][:, 0:64],
                                            d_["invk"][ci][:], None, ALU.mult)
                    nc.vector.tensor_scalar(kv_tm[:, 64:128], d_["psvt"][ci][:, 0:64],
                                            d_["ab_tm"][ci][:, 2:3], None, ALU.mult)
                for ci in grp:
                    pskf = step("pskf", ci, ps_sm, [128, 128], f16, "ps")
                    nc.tensor.transpose(pskf[0:64, :], d_["kvtm"][ci][:, 0:64], iden16_sb[:])
                for ci in grp:
                    nc.vector.tensor_copy(step("kfn", ci, p_kv, [64, 128], f16)[:],
                                          d_["pskf"][ci][0:64, :])
                for ci in grp:
                    kw = step("kw", ci, p_kv, [128, 128], f16)
                    for h in range(2):
                        nc.vector.tensor_scalar(kw[:, h * 64:(h + 1) * 64],
                                                d_["kvtm"][ci][:, 0:64],
                                                d_["wend"][ci][:, h:h + 1], None, ALU.mult)
                # state chain (serial across chunks, cheap)
                for ci in grp:
                    hm_new, hm16_new = [], []
                    for h in range(2):
                        ps_hm = ps_sm.tile([128, 128], f32, tag="ps", name=f"pshm{ci}_{h}")
                        nc.tensor.matmul(ps_hm[0:64, :], d_["kw"][ci][:, h * 64:(h + 1) * 64],
                                         d_["kvtm"][ci][:], start=True, stop=(ci == 0))
                        if ci > 0:
                            diag = p_hm.tile([64, 64], f32, tag="diag", name=f"diag{ci}_{h}")
                            nc.vector.tensor_scalar(diag[:], iden_sb[0:64, 0:64],
                                                    d_["gbL"][ci][:, h:h + 1], None, ALU.mult)
                            nc.tensor.matmul(ps_hm[0:64, :], diag[:], hm_prev[h][:],
                                             start=False, stop=True)
                        hm = p_hm.tile([64, 128], f32, tag="hm", name=f"hm{ci}_{h}")
                        nc.vector.tensor_copy(hm[:], ps_hm[0:64, :])
                        hm16 = p_hm.tile([64, 128], f16, tag="hm16", name=f"hm16_{ci}_{h}")
                        nc.gpsimd.tensor_copy(hm16[:], hm[:])
                        hm_new.append(hm)
                        hm16_new.append(hm16)
                        if dbg and _rep == 0:
                            nc.sync.dma_start(out=dhm[ci, h], in_=hm[:])
                    if dbg and _rep == 0:
                        nc.sync.dma_start(out=dkvtm[ci], in_=d_["kvtm"][ci][:])
                        nc.sync.dma_start(out=dgt[ci], in_=d_["gt"][ci][:])
                    hm16_states.append(hm16_prev)
                    hm16_prev = hm16_new
                    hm_prev = hm_new
                    chunk_ctx.append((d_["kvtm"][ci], d_["kfn"][ci], d_["gt"][ci],
                                      d_["gb"][ci], d_["ab_tm"][ci]))

            # ---- pass 2: operator applications + output, emitted step-major
            # over groups of 4 chunks so the static schedule pipelines the
            # cross-engine chains (PE mm -> DVE mask -> PE mm -> DVE combine)
            for grp in (range(0, 4), range(4, 8)):
                grp = list(grp)
                xcur = {ci: qst[:, ci * 256:(ci + 1) * 256] for ci in grp}
                for it in range(3):          # it 0,1: H-refine; it 2: M-output
                    xg, ps_p, a_sb = {}, {}, {}
                    for ci in grp:
                        if ci > 0:
                            xg[ci] = p_xq.tile([64, 256], f16, tag="xg", name=f"xg{ci}")
                            nc.vector.tensor_tensor(xg[ci][:], xcur[ci],
                                                    chunk_ctx[ci][3][:], ALU.mult)
                    for ci in grp:
                        ps_p[ci] = ps_big.tile([128, 256], f32, tag="ps", name=f"psp{ci}")
                        nc.tensor.matmul(ps_p[ci][:], chunk_ctx[ci][1][:], xcur[ci],
                                         start=True, stop=True)
                    for ci in grp:
                        a_sb[ci] = p_big.tile([128, 256], f16, tag="a", name=f"asb{ci}")
                        nc.vector.tensor_tensor(a_sb[ci][:], ps_p[ci][:],
                                                chunk_ctx[ci][2][:], ALU.mult)
                    if it < 2:
                        ps_y = {}
                        for ci in grp:
                            kv_tm = chunk_ctx[ci][0]
                            ps_y[ci] = ps_med.tile([64, 256], f32, tag="ps", name=f"psy{ci}")
                            nc.tensor.matmul(ps_y[ci][:], kv_tm[:, 0:64], a_sb[ci][:],
                                             start=True, stop=(ci == 0))
                            if ci > 0:
                                for h in range(2):
                                    nc.tensor.matmul(
                                        ps_y[ci][:, h * L:(h + 1) * L],
                                        hm16_states[ci][h][:, 0:64],
                                        xg[ci][:, h * L:(h + 1) * L],
                                        start=False, stop=True)
                        for ci in grp:
                            xq = p_xq.tile([64, 256], f16, tag="xq")
                            nc.vector.scalar_tensor_tensor(
                                xq[:], ps_y[ci][:], -CHEB_DAMP,
                                qst[:, ci * 256:(ci + 1) * 256],
                                op0=ALU.mult, op1=ALU.add)
                            xcur[ci] = xq[:]
                    else:
                        ps_o = {}
                        for ci in grp:
                            kv_tm = chunk_ctx[ci][0]
                            ps_o[ci] = ps_sm.tile([128, 128], f32, tag="ps", name=f"pso{ci}")
                            for h in range(2):
                                nc.tensor.matmul(
                                    ps_o[ci][:, h * 64:(h + 1) * 64],
                                    a_sb[ci][:, h * L:(h + 1) * L], kv_tm[:, 64:128],
                                    start=True, stop=(ci == 0))
                                if ci > 0:
                                    nc.tensor.matmul(
                                        ps_o[ci][:, h * 64:(h + 1) * 64],
                                        xg[ci][:, h * L:(h + 1) * L],
                                        hm16_states[ci][h][:, 64:128],
                                        start=False, stop=True)

                # ---- alpha gate + per-head rmsnorm (time-major), step-major ----
                oa, sqo, sso, lno, rmso, invo, on, ofm = {}, {}, {}, {}, {}, {}, {}, {}
                for ci in grp:
                    al2 = chunk_ctx[ci][4][:, 0:2].unsqueeze(2).broadcast_to([128, 2, 64])
                    oa[ci] = p_out.tile([128, 128], f32, tag="oa", name=f"oa{ci}")
                    nc.vector.tensor_tensor(oa[ci][:].rearrange("p (h v) -> p h v", h=2),
                                            ps_o[ci][:].rearrange("p (h v) -> p h v", h=2),
                                            al2, ALU.mult)
                for ci in grp:
                    sqo[ci] = p_out.tile([128, 128], f32, tag="sqo", name=f"sqo{ci}")
                    nc.scalar.activation(sqo[ci][:], oa[ci][:], AF.Square)
                for ci in grp:
                    sso[ci] = p_sm.tile([128, 2], f32, tag="sso", name=f"sso{ci}")
                    nc.vector.tensor_reduce(sso[ci][:],
                                            sqo[ci][:].rearrange("p (h v) -> p h v", h=2),
                                            X, ALU.add)
                for ci in grp:
                    lno[ci] = p_sm.tile([128, 2], f32, tag="lno", name=f"lno{ci}")
                    nc.scalar.activation(lno[ci][:], sso[ci][:], AF.Ln, bias=epsb[:],
                                         scale=1.0 / 64.0)
                for ci in grp:
                    rmso[ci] = p_sm.tile([128, 2], f32, tag="rmso", name=f"rmso{ci}")
                    nc.scalar.activation(rmso[ci][:], lno[ci][:], AF.Exp, scale=0.5)
                for ci in grp:
                    invo[ci] = p_sm.tile([128, 2], f32, tag="invo", name=f"invo{ci}")
                    nc.vector.reciprocal(invo[ci][:], rmso[ci][:])
                for ci in grp:
                    on[ci] = p_out.tile([128, 128], f16, tag="on", name=f"on{ci}")
                    nc.vector.tensor_tensor(
                        on[ci][:].rearrange("p (h v) -> p h v", h=2),
                        oa[ci][:].rearrange("p (h v) -> p h v", h=2),
                        invo[ci][:].unsqueeze(2).broadcast_to([128, 2, 64]), ALU.mult)
                    if dbg and _rep == 0:
                        nc.sync.dma_start(out=don[ci], in_=on[ci][:])
                ps_of, ps_out = {}, {}
                for ci in grp:
                    ps_of[ci] = ps_sm.tile([128, 128], f16, tag="ps", name=f"psof{ci}")
                    nc.tensor.transpose(ps_of[ci][:], on[ci][:], iden16_sb[:])
                for ci in grp:
                    ofm[ci] = p_out.tile([128, 128], f16, tag="ofm", name=f"ofm{ci}")
                    nc.vector.tensor_copy(ofm[ci][:], ps_of[ci][:])
                for ci in grp:
                    out_sb = p_out.tile([128, D], f32, tag="outsb")
                    for nh in range(2):
                        ps_out = ps_pj.tile([128, 512], f32, tag="ps")
                        nc.tensor.matmul(ps_out[:], ofm[ci][:],
                                         wo_sb[:, nh * 512:(nh + 1) * 512],
                                         start=True, stop=True)
                        nc.scalar.copy(out_sb[:, nh * 512:(nh + 1) * 512], ps_out[:])
                    nc.sync.dma_start(out=outp[ci * L:(ci + 1) * L, :], in_=out_sb[:])

    # The act-table placement pass maps each activation func to the FIRST
    # table containing it; Exp->exp_and_others and Ln->natural_log would then
    # thrash with a table reload on every Exp<->Ln alternation. Compile with
    # natural_log_exp_and_others (has both) hoisted to the front, then remap
    # the emitted set ids back to the real act_info.json indices.
    import concourse.bacc as bacc_mod
    from concourse.hw_specs import get_activation_tables as _gat
    orig_tables = _gat(nc.m.arch)
    orig_names = list(orig_tables.keys())
    pref = "natural_log_exp_and_others"
    reordered = {pref: orig_tables[pref],
                 **{k: v for k, v in orig_tables.items() if k != pref}}
    pnames = list(reordered.keys())
    bacc_mod.get_activation_tables = lambda arch: reordered
    try:
        nc.compile()
    finally:
        bacc_mod.get_activation_tables = _gat
    for b in nc.main_func.blocks:
        for i in b.instructions:
            if isinstance(i, mybir.InstLoadActFuncSet):
                i.act_func_set_id = orig_names.index(pnames[i.act_func_set_id])
    return nc


def _prep_core_inputs(c, x, Wq, Wk, Wv, Wconv, Wa, Walpha, Wb, A_log, dt_bias,
                      norm_w, Wo, xT, xT16, iden, iden16, um, nm):
    f32, f16 = np.float32, np.float16
    h0, h1, hk = 2 * c, 2 * c + 1, c // 2
    wbase = np.hstack([
        Wq[:, h0 * HK:(h0 + 1) * HK], Wq[:, h1 * HK:(h1 + 1) * HK],
        Wk[:, hk * HK:(hk + 1) * HK], Wv[:, hk * HV:(hk + 1) * HV],
    ]).astype(f32)
    wgm = np.hstack([
        Wa[:, h0:h0 + 1], Wa[:, h1:h1 + 1],
        Walpha[:, h0:h0 + 1], Walpha[:, h1:h1 + 1],
        Wb[:, hk:hk + 1],
    ]).astype(f16)
    qoff, koff, voff = 0, HQ * HK, HQ * HK + HKV * HK
    wcv = np.vstack([
        Wconv[qoff + h0 * HK: qoff + (h0 + 1) * HK],
        Wconv[qoff + h1 * HK: qoff + (h1 + 1) * HK],
        Wconv[koff + hk * HK: koff + (hk + 1) * HK],
        Wconv[voff + hk * HV: voff + (hk + 1) * HV],
    ]).astype(f32)
    wcat = wbase.astype(f16)
    wo_scale = np.tile(np.asarray(norm_w, f32), HQ)
    Wo_s = np.asarray(Wo, f32) * wo_scale[:, None]
    wo = np.ascontiguousarray(
        np.vstack([Wo_s[h0 * HV:(h0 + 1) * HV], Wo_s[h1 * HV:(h1 + 1) * HV]])).astype(f16)
    alog = np.asarray(A_log, f32)[[h0, h1]].reshape(1, 2).copy()
    dtbv = np.zeros((1, 5), np.float16)
    dtbv[0, 0:2] = np.asarray(dt_bias, f32)[[h0, h1]]
    return dict(xT16=xT16, wcat=np.ascontiguousarray(wcat), wg=wgm,
                wo=wo, wcv=np.ascontiguousarray(wcv), alog=alog, dtb5=dtbv,
                iden=iden, iden16=iden16, umask=um, nmask=nm)


def make_in_maps(x, Wq, Wk, Wv, Wconv, Wa, Walpha, Wb, A_log, dt_bias, norm_w, Wo):
    f32, f16 = np.float32, np.float16
    x2 = np.asarray(x, f32).reshape(T, D)
    xT = np.ascontiguousarray(x2.T)
    xT16 = xT.astype(f16)
    iden = np.eye(128, dtype=f32)
    iden16 = np.eye(128, dtype=f16)
    um = np.ascontiguousarray(np.triu(np.ones((128, 128), f32)))
    nm = np.ascontiguousarray(np.where(um > 0, 0.0, -30000.0).astype(f32))
    args = (x, np.asarray(Wq, f32), np.asarray(Wk, f32), np.asarray(Wv, f32),
            np.asarray(Wconv, f32), np.asarray(Wa, f32), np.asarray(Walpha, f32),
            np.asarray(Wb, f32), A_log, dt_bias, norm_w, Wo)
    return [_prep_core_inputs(c, *args, xT=xT, xT16=xT16, iden=iden,
                              iden16=iden16, um=um, nm=nm)
            for c in range(NCORES)]


def get_program(dbg=False, reps=1):
    key = (dbg, reps)
    if key not in _PROG_CACHE:
        _PROG_CACHE[key] = _build_program(dbg, reps)
    return _PROG_CACHE[key]


def kernel(**inputs) -> np.ndarray:
    from concourse.bass_utils import run_bass_kernel_spmd
    nc = get_program(dbg=False)
    in_maps = make_in_maps(**inputs)
    res = run_bass_kernel_spmd(nc, in_maps, list(range(NCORES)))
    out = np.zeros((T, D), np.float32)
    for c in range(NCORES):
        out += res.results[c]["outp"]
    return out.reshape(B, T, D)



# revision 28
# speedup vs baseline: 7.6077x; 7.6077x over previous
"""Trainium2 Bass kernel for the PrimedGKA layer (gated linear attention with
Chebyshev query refinement), tensor-parallel over the 16 query heads across
8 NeuronCores (2 q-heads + their shared kv-head per core); per-core partial
out-projections are summed on the host.

Restructured vs the baseline to relieve the DVE and PE-sequencer bottlenecks:
feature-major gate projection transposed once via the DMA XBAR, one batched
cumsum matmul for all chunks' decay logs, decay masks via Activation-engine
exp with a per-partition -G_s bias (PE matmul supplies the G_t broadcast and
the -30000 causal clamp), the causal conv as 4 accumulating diagonal matmuls
on PE, raw-k score matmuls with 1/||k|| folded into the query/value sides,
head-stacked 128-partition pass-2 layout so the per-head recurrent-state
matmuls merge into block-diagonal single matmuls, and elementwise traffic
split across DVE / GpSimd / Act.

Self-contained: hardcodes all shapes from the problem spec.
"""
import numpy as np

B, T, D = 1, 1024, 1024
HQ, HKV, HK, HV = 16, 4, 64, 64
KW = 4
NCORES = 8
L = 128                 # chunk length
NCH = T // L            # 8 chunks
CHEB_DAMP = 0.25
EPS = 1e-6

_PROG_CACHE = {}


def _build_program(dbg=False, reps=1):
    import concourse.bacc as bacc
    import concourse.mybir as mybir
    from concourse.tile import TileContext

    dt = mybir.dt
    f32 = dt.float32
    f32r = dt.float32r
    f16 = dt.float16
    AF = mybir.ActivationFunctionType
    ALU = mybir.AluOpType
    X = mybir.AxisListType.X

    nc = bacc.Bacc("TRN2", target_bir_lowering=False, debug=False,
                   num_devices=NCORES)

    xT16 = nc.dram_tensor("xT16", [D, T], f16, kind="ExternalInput")
    wcat = nc.dram_tensor("wcat", [D, 256], f16, kind="ExternalInput")
    wg5 = nc.dram_tensor("wg5", [D, 5], f16, kind="ExternalInput")
    convd = nc.dram_tensor("convd", [128, 8 * 128], f16, kind="ExternalInput")
    wo = nc.dram_tensor("wo", [128, D], f16, kind="ExternalInput")
    alog = nc.dram_tensor("alog", [1, 2], f32, kind="ExternalInput")
    dtbbc = nc.dram_tensor("dtbbc", [128, 2], f32, kind="ExternalInput")
    um = nc.dram_tensor("um", [128, 128], f32, kind="ExternalInput")
    up16 = nc.dram_tensor("up16", [128, 128], f16, kind="ExternalInput")
    rhs30k = nc.dram_tensor("rhs30k", [128, 512], f16, kind="ExternalInput")
    iden = nc.dram_tensor("iden", [128, 128], f16, kind="ExternalInput")
    selall = nc.dram_tensor("selall", [16, 1024], f16, kind="ExternalInput")
    wcvq = nc.dram_tensor("wcvq", [128, 4], f32, kind="ExternalInput")
    outp = nc.dram_tensor("outp", [T, D], f16, kind="ExternalOutput")
    if dbg:
        dSq = nc.dram_tensor("dSq", [128, T], f16, kind="ExternalOutput")
        dSkv = nc.dram_tensor("dSkv", [128, T], f16, kind="ExternalOutput")
        dgtm = nc.dram_tensor("dgtm", [128, 64], f16, kind="ExternalOutput")
        dgt = nc.dram_tensor("dgt", [128, 2048], f16, kind="ExternalOutput")
        dgb = nc.dram_tensor("dgb", [128, 1024], f16, kind="ExternalOutput")
        dkvtm = nc.dram_tensor("dkvtm", [8, 128, 192], f16, kind="ExternalOutput")
        dhmk = nc.dram_tensor("dhmk", [8, 128, 128], f16, kind="ExternalOutput")
        dhmv = nc.dram_tensor("dhmv", [8, 128, 128], f16, kind="ExternalOutput")
        dqse = nc.dram_tensor("dqse", [128, 2048], f16, kind="ExternalOutput")
        dxce = nc.dram_tensor("dxce", [8, 128, 256], f16, kind="ExternalOutput")
        dsfin = nc.dram_tensor("dsfin", [128, 16], f32, kind="ExternalOutput")
        dinvk = nc.dram_tensor("dinvk", [128, 8], f32, kind="ExternalOutput")

    with TileContext(nc) as tc:
      import contextlib
      for _rep in range(reps):
        ctx = contextlib.ExitStack()
        with ctx:
            pers = ctx.enter_context(tc.tile_pool(name="pers", bufs=1))
            p_sm = ctx.enter_context(tc.tile_pool(name="p_sm", bufs=9))
            p_kv = ctx.enter_context(tc.tile_pool(name="p_kv", bufs=9))
            p_xq = ctx.enter_context(tc.tile_pool(name="p_xq", bufs=10))
            p_out = ctx.enter_context(tc.tile_pool(name="p_out", bufs=6))
            ps_all = ctx.enter_context(tc.tile_pool(name="ps_all", bufs=6, space="PSUM"))
            ps_oy = ctx.enter_context(tc.tile_pool(name="ps_oy", bufs=2, space="PSUM"))

            # ================= persistent loads =================
            wcat_sb = pers.tile([128, 8, 256], f16)
            nc.sync.dma_start(out=wcat_sb[:], in_=wcat[:].rearrange("(a p) c -> p a c", p=128))
            wg5_sb = pers.tile([128, 8, 5], f16)
            nc.sync.dma_start(out=wg5_sb[:], in_=wg5[:].rearrange("(a p) c -> p a c", p=128))
            convd_sb = pers.tile([128, 8, 128], f16)
            nc.scalar.dma_start(out=convd_sb[:], in_=convd[:].rearrange("p (a c) -> p a c", a=8))
            xt16_sb = pers.tile([128, 8, T], f16)
            for d in range(0, 8, 2):
                eng = nc.sync if d % 4 == 0 else nc.scalar
                eng.dma_start(out=xt16_sb[:, d:d + 2, :],
                              in_=xT16[d * 128:(d + 2) * 128, :]
                              .rearrange("(a p) c -> p a c", p=128))
            wo_sb = pers.tile([128, D], f16)
            nc.scalar.dma_start(out=wo_sb[:], in_=wo[:])
            alog_sb = pers.tile([1, 2], f32)
            nc.sync.dma_start(out=alog_sb[:], in_=alog[:])
            dtb_sb = pers.tile([128, 2], f32)
            nc.sync.dma_start(out=dtb_sb[:], in_=dtbbc[:])
            um_sb = pers.tile([128, 128], f32)
            nc.sync.dma_start(out=um_sb[:], in_=um[:])
            up16_sb = pers.tile([128, 128], f16)
            nc.scalar.dma_start(out=up16_sb[:], in_=up16[:])
            rhs30k_sb = pers.tile([128, 512], f16)
            nc.scalar.dma_start(out=rhs30k_sb[:], in_=rhs30k[:])
            iden16_sb = pers.tile([128, 128], f16)
            nc.sync.dma_start(out=iden16_sb[:], in_=iden[:])
            selall_sb = pers.tile([16, 1024], f16)
            nc.scalar.dma_start(out=selall_sb[:], in_=selall[:])
            wcvq_sb = pers.tile([128, 4], f32)
            nc.sync.dma_start(out=wcvq_sb[:], in_=wcvq[:])

            ones128h = pers.tile([1, 128], f16)
            nc.vector.memset(ones128h[:], 1.0)

            # persistent work tiles
            Rq = pers.tile([128, 3 + T], f16)     # padded raw q proj, fm
            Rkv = pers.tile([128, 3 + T], f16)    # padded raw k|v proj, fm
            nc.vector.memset(Rq[:, 0:3], 0.0)
            nc.vector.memset(Rkv[:, 0:3], 0.0)
            Sq = pers.tile([128, T], f16)         # silu(conv(q)), fm (h-stacked)
            Skv = pers.tile([128, T], f16)        # silu(conv(k|v)), fm
            kst = pers.tile([128, T], f16)        # [k ; k] duplicated rows
            gF = pers.tile([5, T], f16)           # feature-major gates
            g_tmT = pers.tile([128, NCH * 8], f16)    # time-major gates (c)(8)
            e_a = pers.tile([128, 16], f32)
            sp_a = pers.tile([128, 16], f32)
            g_all = pers.tile([128, 16], f32)
            e_ab = pers.tile([128, 24], f32)
            d_ab = pers.tile([128, 24], f32)
            ab_all = pers.tile([128, 24], f32)    # (c)(al0, al1, beta) sigmoids
            asq = pers.tile([128, 16], f32)       # alpha^2 (c)(h)
            negG = pers.tile([128, 16], f32)
            growall = pers.tile([16, 128], f32)
            growhi = pers.tile([16, 128], f16)
            growres = pers.tile([16, 128], f16)
            growrowh = pers.tile([1, 2048], f16)
            growrowr = pers.tile([1, 2048], f16)
            Cq = pers.tile([128, T], f16)
            gamlog16 = pers.tile([16, 128], f16)
            gt_all = pers.tile([128, 2048], f16)  # decay masks (c)(h)(t)
            gb_st = pers.tile([128, 1024], f16)   # gamma, head-stacked (c)(t)
            sqk_all = pers.tile([128, 512], f16)
            ssk = pers.tile([128, 8], f32)
            lnk = pers.tile([128, 8], f32)
            invk = pers.tile([128, 8], f32)
            sq_all = pers.tile([128, 1024], f16)  # rms squares (c)(h*64)
            sso = pers.tile([128, 16], f32)
            uvar = pers.tile([128, 16], f32)
            lno = pers.tile([128, 16], f32)
            sfac = pers.tile([128, 16], f32)
            sfin = pers.tile([128, 16], f32)
            era = pers.tile([1, 2], f16)
            negea = pers.tile([128, 2], f32)
            glf = pers.tile([128, 8], f32)        # gammaL per chunk, h-stacked
            b1em12 = pers.tile([128, 1], f32)
            nc.vector.memset(b1em12[:], 1e-12)
            bepsk = pers.tile([128, 1], f32)
            nc.vector.memset(bepsk[:], EPS * HK)

            # q in block-diagonal stacked layout: rows 0:64 head0 (cols c*256
            # .. +128), rows 64:128 head1 (cols c*256+128 .. +256), 0 elsewhere
            qse = pers.tile([128, 2048], f16)
            nc.vector.memset(qse[0:64, :].rearrange("p (c a t) -> p c a t", c=NCH, a=2)[:, :, 1, :], 0.0)
            nc.vector.memset(qse[64:128, :].rearrange("p (c a t) -> p c a t", c=NCH, a=2)[:, :, 0, :], 0.0)

            kvt = [pers.tile([128, 128], f16, name=f"kvt{c}") for c in range(NCH)]
            kvtm = [pers.tile([128, 192], f16, name=f"kvtm{c}") for c in range(NCH)]
            kw = [pers.tile([128, 128], f16, name=f"kw{c}") for c in range(NCH)]
            knegp = [pers.tile([128, 256], f16, name=f"knegp{c}") for c in range(NCH)]
            hmk = [pers.tile([128, 128], f16, name=f"hmk{c}") for c in range(NCH)]
            hmv = [pers.tile([128, 128], f16, name=f"hmv{c}") for c in range(NCH)]
            xce = [pers.tile([128, 256], f16, name=f"xce{c}") for c in range(NCH)]
            for c in range(NCH):
                # zero-pad: knegp halves, hm block-diag off-blocks, xce quadrants
                nc.gpsimd.memset(knegp[c][:, 64:192], 0.0)
                if c > 0:
                    nc.gpsimd.memset(hmk[c][0:64, 64:128], 0.0)
                    nc.gpsimd.memset(hmk[c][64:128, 0:64], 0.0)
                    nc.gpsimd.memset(hmv[c][0:64, 64:128], 0.0)
                    nc.gpsimd.memset(hmv[c][64:128, 0:64], 0.0)
                nc.vector.memset(xce[c][0:64, 128:256], 0.0)
                nc.vector.memset(xce[c][64:128, 0:128], 0.0)

            # ============ front: projections / gates / conv, interleaved ====
            # ordering tuned so each engine queue receives work in expected
            # readiness order (in-order queues suffer head-of-line blocking)
            proj_ps = {}

            def proj_mms(ct, dlist):
                c0 = ct * 128
                if ct not in proj_ps:
                    proj_ps[ct] = [ps_all.tile([128, 512], f32, tag="ps",
                                               name=f"pj{ct}{th}") for th in range(2)]
                for d in dlist:
                    for th in range(2):
                        nc.tensor.matmul(proj_ps[ct][th][:], wcat_sb[:, d, c0:c0 + 128],
                                         xt16_sb[:, d, th * 512:(th + 1) * 512],
                                         start=(d == 0), stop=(d == 7))

            def proj_copies(ct, R):
                for th in range(2):
                    dst = R[:, 3 + th * 512: 3 + (th + 1) * 512]
                    if th == 0:
                        nc.vector.tensor_copy(dst, proj_ps[ct][th][:])
                    else:
                        nc.scalar.copy(dst, proj_ps[ct][th][:])

            def conv_mms(tile_idx, R):
                cps = [ps_all.tile([128, 512], f32, tag="ps", name=f"cv{tile_idx}{th}")
                       for th in range(2)]
                for i in range(KW):
                    for th in range(2):
                        nc.tensor.matmul(cps[th][:],
                                         convd_sb[:, tile_idx * 4 + i, :],
                                         R[:, i + th * 512: i + th * 512 + 512],
                                         start=(i == 0), stop=(i == KW - 1))
                return cps

            # -- kv path first: projection, conv, silu, transposes --
            proj_mms(1, range(0, 8))
            proj_copies(1, Rkv)
            cps_kv = conv_mms(1, Rkv)
            for th in range(2):
                nc.scalar.activation(Skv[:, th * 512:(th + 1) * 512], cps_kv[th][:], AF.Silu)
            nc.sync.dma_start(out=kst[0:64, :], in_=Skv[0:64, :])
            nc.scalar.dma_start(out=kst[64:128, :], in_=Skv[0:64, :])
            for c in range(NCH):
                eng = nc.sync if c % 2 == 0 else nc.scalar
                eng.dma_start(out=kvt[c][:], in_=Skv[:, c * L:(c + 1) * L],
                              transpose=True)

            # -- gate projection + transposes --
            pg = [ps_all.tile([5, 512], f32, tag="ps", name=f"pg{th}") for th in range(2)]
            for d in range(8):
                for th in range(2):
                    nc.tensor.matmul(pg[th][:], wg5_sb[:, d, :],
                                     xt16_sb[:, d, th * 512:(th + 1) * 512],
                                     start=(d == 0), stop=(d == 7))
            for th in range(2):
                nc.scalar.copy(gF[:, th * 512:(th + 1) * 512], pg[th][:])
            ps_gt = ps_all.tile([128, 64], f16, tag="ps", name="psgt")
            for c in range(NCH):
                nc.tensor.transpose(ps_gt[:, c * 8:c * 8 + 5],
                                    gF[:, c * 128:(c + 1) * 128],
                                    iden16_sb[0:5, 0:5])
            nc.vector.tensor_copy(g_tmT[:], ps_gt[:])

            # -- gate chain (Act/DVE) --
            gv = g_tmT[:].rearrange("p (c r) -> p c r", r=8)
            eav = e_a[:].rearrange("p (c h) -> p c h", h=2)
            for h in range(2):
                nc.scalar.activation(eav[:, :, h], gv[:, :, h], AF.Exp,
                                     bias=dtb_sb[:, h:h + 1])
            nc.scalar.activation(sp_a[:], e_a[:], AF.Ln, bias=1.0)
            nc.scalar.activation(era[:], alog_sb[:], AF.Exp)
            ps_bc = ps_all.tile([128, 128], f32, tag="ps", name="psbc")
            nc.tensor.matmul(ps_bc[:, 0:2], ones128h[:], era[:], start=True, stop=True)
            nc.vector.tensor_scalar(negea[:], ps_bc[:, 0:2], -1.0, None, ALU.mult)
            spv = sp_a[:].rearrange("p (c h) -> p c h", h=2)
            gav = g_all[:].rearrange("p (c h) -> p c h", h=2)
            for h in range(2):
                nc.vector.tensor_scalar(gav[:, :, h], spv[:, :, h],
                                        negea[:, h:h + 1], None, ALU.mult)
            nc.scalar.activation(e_ab[:].rearrange("p (c r) -> p c r", r=3),
                                 gv[:, :, 2:5], AF.Exp, scale=-1.0)
            nc.vector.tensor_scalar(d_ab[:], e_ab[:], 1.0, None, ALU.add)
            nc.vector.reciprocal(ab_all[:], d_ab[:])
            nc.scalar.activation(asq[:].rearrange("p (c h) -> p c h", h=2),
                                 ab_all[:].rearrange("p (c r) -> p c r", r=3)[:, :, 0:2],
                                 AF.Square)

            # -- k-norm squares + per-pair invk (Act ops early in queue) --
            for c in range(NCH):
                nc.vector.tensor_tensor(sqk_all[:, c * 64:(c + 1) * 64],
                                        kvt[c][:, 0:64], kvt[c][:, 0:64], ALU.mult)
                if c % 2 == 1:
                    nc.vector.tensor_reduce(
                        ssk[:, c - 1:c + 1],
                        sqk_all[:, (c - 1) * 64:(c + 1) * 64]
                        .rearrange("p (c k) -> p c k", c=2), X, ALU.add)
                    nc.scalar.activation(lnk[:, c - 1:c + 1], ssk[:, c - 1:c + 1],
                                         AF.Ln, bias=b1em12[:])
                    nc.scalar.activation(invk[:, c - 1:c + 1], lnk[:, c - 1:c + 1],
                                         AF.Exp, scale=-0.5)

            # -- q path (cumsum matmuls interleaved into the PE stream) --
            proj_mms(0, range(0, 4))
            psG = ps_all.tile([128, 16], f32, tag="ps", name="psG")
            nc.tensor.matmul(psG[:], um_sb[:], g_all[:], start=True, stop=True)
            nc.vector.tensor_scalar(negG[:], psG[:], -1.0, None, ALU.mult)
            psGr = ps_all.tile([16, 128], f32, tag="ps", name="psGr")
            nc.tensor.matmul(psGr[:], g_all[:], um_sb[:], start=True, stop=True)
            nc.vector.tensor_copy(growall[:], psGr[:])
            nc.vector.tensor_copy(growhi[:], psGr[:])
            nc.vector.tensor_tensor(growres[:], growall[:], growhi[:], ALU.subtract)
            nc.sync.dma_start(out=growrowh[:], in_=growhi[:])
            nc.scalar.dma_start(out=growrowr[:], in_=growres[:])
            nc.vector.tensor_copy(gamlog16[:], psGr[:])
            proj_mms(0, range(4, 8))
            proj_copies(0, Rq)
            cps_q = conv_mms(0, Rq)
            for th in range(2):
                nc.scalar.activation(Sq[:, th * 512:(th + 1) * 512], cps_q[th][:], AF.Silu)

            # gamma broadcast (log-domain selector matmul, exp) - before masks
            for b in range(2):
                psgb = ps_all.tile([128, 512], f32, tag="ps", name=f"psgb{b}")
                for k in range(4):
                    c = b * 4 + k
                    nc.tensor.matmul(psgb[:, k * 128:(k + 1) * 128],
                                     selall_sb[:, c * 128:(c + 1) * 128],
                                     gamlog16[:], start=True, stop=True)
                for k in range(2):
                    nc.scalar.activation(gb_st[:, b * 512 + k * 256: b * 512 + (k + 1) * 256],
                                         psgb[:, k * 256:(k + 1) * 256], AF.Exp)
            nc.vector.tensor_copy(
                glf[:], gb_st[:].rearrange("p (c t) -> p c t", c=NCH)[:, :, L - 1])

            # decay masks
            for b in range(4):
                psGB = ps_all.tile([128, 512], f32, tag="ps", name=f"psGB{b}")
                nc.tensor.matmul(psGB[:], ones128h[:], growrowh[:, b * 512:(b + 1) * 512],
                                 start=True, stop=False)
                nc.tensor.matmul(psGB[:], ones128h[:], growrowr[:, b * 512:(b + 1) * 512],
                                 start=False, stop=False)
                nc.tensor.matmul(psGB[:], up16_sb[:], rhs30k_sb[:],
                                 start=False, stop=True)
                for k in range(4):
                    r = b * 4 + k
                    nc.scalar.activation(gt_all[:, r * 128:(r + 1) * 128],
                                         psGB[:, k * 128:(k + 1) * 128], AF.Exp,
                                         bias=negG[:, r:r + 1])

            # ============ kvtm + kw + state chain (per chunk) ======
            hm_prev = None
            for c in range(NCH):
                km = kvtm[c]
                nc.vector.tensor_scalar(km[:, 0:64], kvt[c][:, 0:64],
                                        invk[:, c:c + 1], None, ALU.mult)
                nc.vector.tensor_scalar(km[:, 64:128], kvt[c][:, 64:128],
                                        ab_all[:, c * 3 + 2:c * 3 + 3], None, ALU.mult)
                nc.vector.tensor_scalar(km[:, 128:192], km[:, 64:128],
                                        invk[:, c:c + 1], None, ALU.mult)
                nc.gpsimd.tensor_scalar(knegp[c][:, 0:64], km[:, 0:64],
                                        invk[:, c:c + 1], -CHEB_DAMP, ALU.mult, ALU.mult)
                nc.gpsimd.tensor_copy(knegp[c][:, 192:256], knegp[c][:, 0:64])
                for h in range(2):
                    nc.vector.tensor_tensor(
                        kw[c][:, h * 64:(h + 1) * 64], km[:, 0:64],
                        gt_all[:, c * 256 + h * 128 + L - 1: c * 256 + h * 128 + L]
                        .broadcast_to([128, 64]), ALU.mult)
                ps_hm = ps_all.tile([128, 128], f32, tag="ps", name=f"pshm{c}")
                nc.tensor.matmul(ps_hm[:], kw[c][:], kvtm[c][:, 0:128],
                                 start=True, stop=(c == 0))
                if c > 0:
                    dg = p_sm.tile([128, 128], f16, tag="diag", name=f"diag{c}")
                    nc.vector.tensor_scalar(dg[:], iden16_sb[:],
                                            glf[:, c:c + 1], None, ALU.mult)
                    nc.tensor.matmul(ps_hm[:], dg[:], hm_prev[:],
                                     start=False, stop=True)
                if c + 1 < NCH:
                    hm = p_kv.tile([128, 128], f16, tag="hm", name=f"hm{c}")
                    nc.vector.tensor_copy(hm[:], ps_hm[:])
                    hm_prev = hm
                    cn = c + 1
                    nc.vector.tensor_scalar(hmk[cn][0:64, 0:64], ps_hm[0:64, 0:64],
                                            -CHEB_DAMP, None, ALU.mult)
                    nc.vector.tensor_scalar(hmk[cn][64:128, 64:128], ps_hm[64:128, 0:64],
                                            -CHEB_DAMP, None, ALU.mult)
                    nc.scalar.copy(hmv[cn][0:64, 0:64], ps_hm[0:64, 64:128])
                    nc.scalar.copy(hmv[cn][64:128, 64:128], ps_hm[64:128, 64:128])

            if dbg and _rep == 0:
                nc.sync.dma_start(out=dSq[:], in_=Sq[:])
                nc.sync.dma_start(out=dSkv[:], in_=Skv[:])
                nc.sync.dma_start(out=dgtm[:], in_=g_tmT[:])
                nc.sync.dma_start(out=dgt[:], in_=gt_all[:])
                nc.sync.dma_start(out=dgb[:], in_=gb_st[:])
                nc.sync.dma_start(out=dinvk[:], in_=invk[:])
                for c in range(NCH):
                    nc.sync.dma_start(out=dkvtm[c], in_=kvtm[c][:])
                    if c > 0:
                        nc.sync.dma_start(out=dhmk[c], in_=hmk[c][:])
                        nc.sync.dma_start(out=dhmv[c], in_=hmv[c][:])

            # stacked q into block-diag quadrants (feeds pass-2 scores)
            qv = qse[:].rearrange("p (c a t) -> p c a t", c=NCH, a=2)
            nc.vector.tensor_copy(
                qv[0:64, :, 0, :],
                Sq[0:64, :].rearrange("p (c t) -> p c t", c=NCH))
            nc.vector.tensor_copy(
                qv[64:128, :, 1, :],
                Sq[64:128, :].rearrange("p (c t) -> p c t", c=NCH))

            # ============ pass 2: operator applications ============
            # single 8-wide wave: all chunks advance together through the
            # 3 operator applications, maximizing cross-chunk parallelism
            grp = list(range(NCH))
            xcur = {ci: qse[:, ci * 256:(ci + 1) * 256] for ci in grp}
            asb = {}
            ps_o = {}
            for it in range(3):
                ps_p = {}
                pspb_by = {}
                for ci in grp:
                    if ci % 2 == 0:
                        pspb = ps_all.tile([128, 512], f32, tag="ps", name=f"pspb{ci}")
                    pspb_by[ci] = pspb
                    ps_p[ci] = pspb[:, (ci % 2) * 256:(ci % 2 + 1) * 256]
                    nc.tensor.matmul(ps_p[ci], kst[:, ci * L:(ci + 1) * L],
                                     xcur[ci], start=True, stop=True)
                for ci in grp:
                    if ci % 2 == 1:
                        a2 = p_kv.tile([128, 512], f16, tag="a", name=f"asb{ci}")
                        nc.vector.tensor_tensor(
                            a2[:], pspb_by[ci][:],
                            gt_all[:, (ci - 1) * 256:(ci + 1) * 256], ALU.mult)
                        asb[ci - 1] = a2[:, 0:256]
                        asb[ci] = a2[:, 256:512]
                # gamma-scaled queries (compact stacked layout)
                xgs = {}
                for ci in grp:
                    if ci > 0:
                        xs = p_xq.tile([128, 128], f16, tag="xgs", name=f"xgs{ci}")
                        nc.gpsimd.tensor_tensor(
                            xs[0:64, :], xcur[ci][0:64, 0:128],
                            gb_st[0:64, ci * 128:(ci + 1) * 128], ALU.mult)
                        nc.gpsimd.tensor_tensor(
                            xs[64:128, :], xcur[ci][64:128, 128:256],
                            gb_st[64:128, ci * 128:(ci + 1) * 128], ALU.mult)
                        xgs[ci] = xs
                if it < 2:
                    ps_y = {}
                    for ci in grp:
                        if ci % 4 == 0:
                            psyb = ps_oy.tile([128, 512], f32, tag="psy", name=f"psyb{it}{ci}")
                        ps_y[ci] = psyb[:, (ci % 4) * 128:(ci % 4 + 1) * 128]
                        nc.tensor.matmul(ps_y[ci], knegp[ci][:, 0:128], asb[ci][:, 0:128],
                                         start=True, stop=False)
                        nc.tensor.matmul(ps_y[ci], knegp[ci][:, 128:256], asb[ci][:, 128:256],
                                         start=False, stop=(ci == 0))
                        if ci > 0:
                            nc.tensor.matmul(ps_y[ci], hmk[ci][:], xgs[ci][:],
                                             start=False, stop=True)
                    for ci in grp:
                        xn = xce[ci]
                        nc.vector.tensor_tensor(xn[0:64, 0:128], ps_y[ci][0:64],
                                                qse[0:64, ci * 256:ci * 256 + 128], ALU.add)
                        nc.vector.tensor_tensor(xn[64:128, 128:256], ps_y[ci][64:128],
                                                qse[64:128, ci * 256 + 128:(ci + 1) * 256], ALU.add)
                        xcur[ci] = xn[:]

                else:
                    for ci in grp:
                        if ci % 4 == 0:
                            psob = ps_oy.tile([128, 512], f32, tag="psy", name=f"psob{ci}")
                        ps_o[ci] = psob[:, (ci % 4) * 128:(ci % 4 + 1) * 128]
                        for h in range(2):
                            nc.tensor.matmul(ps_o[ci][:, h * 64:(h + 1) * 64],
                                             asb[ci][:, h * 128:(h + 1) * 128],
                                             kvtm[ci][:, 128:192],
                                             start=True, stop=(ci == 0))
                            if ci > 0:
                                nc.tensor.matmul(ps_o[ci][:, h * 64:(h + 1) * 64],
                                                 xgs[ci][:],
                                                 hmv[ci][:, h * 64:(h + 1) * 64],
                                                 start=False, stop=True)

            if dbg and _rep == 0:
                nc.sync.dma_start(out=dqse[:], in_=qse[:])
                for c in range(NCH):
                    nc.sync.dma_start(out=dxce[c], in_=xce[c][:])

            # ---- rmsnorm (batched) + alpha gate + out projection ----
            for ci in grp:
                nc.scalar.activation(sq_all[:, ci * 128:(ci + 1) * 128],
                                     ps_o[ci], AF.Square)
            nc.vector.tensor_reduce(
                sso[:], sq_all[:].rearrange("p (r k) -> p r k", k=64), X, ALU.add)
            nc.vector.tensor_tensor(uvar[:], sso[:], asq[:], ALU.mult)
            nc.scalar.activation(lno[:], uvar[:], AF.Ln, bias=bepsk[:], scale=1.0 / HV)
            nc.scalar.activation(sfac[:], lno[:], AF.Exp, scale=-0.5)
            nc.vector.tensor_tensor(
                sfin[:].rearrange("p (c h) -> p c h", h=2),
                sfac[:].rearrange("p (c h) -> p c h", h=2),
                ab_all[:].rearrange("p (c r) -> p c r", r=3)[:, :, 0:2],
                ALU.mult)
            if dbg and _rep == 0:
                nc.sync.dma_start(out=dsfin[:], in_=sfin[:])
            out_rr = 0
            for ci in grp:
                o = p_out.tile([128, 128], f16, tag="on", name=f"on{ci}")
                for h in range(2):
                    nc.vector.tensor_scalar(o[:, h * 64:(h + 1) * 64],
                                            ps_o[ci][:, h * 64:(h + 1) * 64],
                                            sfin[:, ci * 2 + h:ci * 2 + h + 1],
                                            None, ALU.mult)
                f = p_out.tile([128, 128], f16, tag="ofm", name=f"ofm{ci}")
                eng = nc.sync if ci % 2 == 0 else nc.scalar
                eng.dma_start(out=f[:], in_=o[:], transpose=True)
                out_sb = p_out.tile([128, D], f16, tag="outsb")
                for nh in range(2):
                    ps_out = ps_all.tile([128, 512], f32, tag="ps", name=f"pso2{ci}{nh}")
                    nc.tensor.matmul(ps_out[:], f[:],
                                     wo_sb[:, nh * 512:(nh + 1) * 512],
                                     start=True, stop=True)
                    rr = out_rr % 2
                    out_rr += 1
                    dst = out_sb[:, nh * 512:(nh + 1) * 512]
                    if rr == 0:
                        nc.scalar.copy(dst, ps_out[:])
                    else:
                        nc.vector.tensor_copy(dst, ps_out[:])
                eng = nc.sync if ci % 2 == 0 else nc.scalar
                eng.dma_start(out=outp[ci * L:(ci + 1) * L, :], in_=out_sb[:])

    # Activation-table placement: map Exp/Ln/Square/Copy into the combined
    # natural_log_exp table (hoisted first) so only Silu forces a switch.
    import concourse.bacc as bacc_mod
    from concourse.hw_specs import get_activation_tables as _gat
    orig_tables = _gat(nc.m.arch)
    orig_names = list(orig_tables.keys())
    pref = "natural_log_exp_and_others"
    reordered = {pref: orig_tables[pref],
                 **{k: v for k, v in orig_tables.items() if k != pref}}
    pnames = list(reordered.keys())
    bacc_mod.get_activation_tables = lambda arch: reordered
    try:
        nc.compile()
    finally:
        bacc_mod.get_activation_tables = _gat
    for b in nc.main_func.blocks:
        for i in b.instructions:
            if isinstance(i, mybir.InstLoadActFuncSet):
                i.act_func_set_id = orig_names.index(pnames[i.act_func_set_id])
    return nc


def _prep_core_inputs(c, x, Wq, Wk, Wv, Wconv, Wa, Walpha, Wb, A_log, dt_bias,
                      norm_w, Wo, xT16, iden16, um32, up16, rhs30k, selall):
    f32, f16 = np.float32, np.float16
    h0, h1, hk = 2 * c, 2 * c + 1, c // 2
    wcat = np.hstack([
        Wq[:, h0 * HK:(h0 + 1) * HK], Wq[:, h1 * HK:(h1 + 1) * HK],
        Wk[:, hk * HK:(hk + 1) * HK], Wv[:, hk * HV:(hk + 1) * HV],
    ]).astype(f16)
    wg5 = np.hstack([
        Wa[:, h0:h0 + 1], Wa[:, h1:h1 + 1],
        Walpha[:, h0:h0 + 1], Walpha[:, h1:h1 + 1],
        Wb[:, hk:hk + 1],
    ]).astype(f16)
    qoff, koff, voff = 0, HQ * HK, HQ * HK + HKV * HK
    wcv = np.vstack([
        Wconv[qoff + h0 * HK: qoff + (h0 + 1) * HK],
        Wconv[qoff + h1 * HK: qoff + (h1 + 1) * HK],
        Wconv[koff + hk * HK: koff + (hk + 1) * HK],
        Wconv[voff + hk * HV: voff + (hk + 1) * HV],
    ]).astype(f32)  # [256, 4]: rows = [q(128) | k(64) | v(64)]
    convd = np.zeros((128, 8, 128), f32)
    for tile in range(2):
        ch = wcv[tile * 128:(tile + 1) * 128]   # [128, KW]
        for tap in range(KW):
            np.fill_diagonal(convd[:, tile * 4 + tap, :], ch[:, tap])
    convd = np.ascontiguousarray(convd.reshape(128, 8 * 128)).astype(f16)
    wcvq = np.ascontiguousarray(wcv[0:128]).astype(f32)
    wo_scale = np.tile(np.asarray(norm_w, f32), HQ)
    Wo_s = np.asarray(Wo, f32) * wo_scale[:, None]
    wo = np.ascontiguousarray(
        np.vstack([Wo_s[h0 * HV:(h0 + 1) * HV], Wo_s[h1 * HV:(h1 + 1) * HV]])).astype(f16)
    alog = np.asarray(A_log, f32)[[h0, h1]].reshape(1, 2).copy()
    dtbbc = np.tile(np.asarray(dt_bias, f32)[[h0, h1]].reshape(1, 2), (128, 1)).copy()
    return dict(xT16=xT16, wcat=np.ascontiguousarray(wcat), wg5=wg5,
                convd=convd, wcvq=wcvq, wo=wo, alog=alog, dtbbc=dtbbc,
                um=um32, up16=up16, rhs30k=rhs30k, iden=iden16, selall=selall)


def make_in_maps(x, Wq, Wk, Wv, Wconv, Wa, Walpha, Wb, A_log, dt_bias, norm_w, Wo):
    f32, f16 = np.float32, np.float16
    x2 = np.asarray(x, f32).reshape(T, D)
    xT16 = np.ascontiguousarray(x2.T).astype(f16)
    iden16 = np.eye(128, dtype=f16)
    um32 = np.ascontiguousarray(np.triu(np.ones((128, 128), f32)))
    up16 = np.ascontiguousarray(np.triu(np.ones((128, 128), f16), 1))
    rhs30k = np.ascontiguousarray(
        np.hstack([-30000.0 * np.eye(128)] * 4)).astype(f16)
    selall = np.zeros((16, 8, 128), f16)
    for c in range(8):
        selall[c * 2, c, 0:64] = 1.0
        selall[c * 2 + 1, c, 64:128] = 1.0
    selall = np.ascontiguousarray(selall.reshape(16, 1024))
    args = (x, np.asarray(Wq, f32), np.asarray(Wk, f32), np.asarray(Wv, f32),
            np.asarray(Wconv, f32), np.asarray(Wa, f32), np.asarray(Walpha, f32),
            np.asarray(Wb, f32), A_log, dt_bias, norm_w, Wo)
    return [_prep_core_inputs(c, *args, xT16=xT16, iden16=iden16, um32=um32,
                              up16=up16, rhs30k=rhs30k, selall=selall)
            for c in range(NCORES)]


def get_program(dbg=False, reps=1):
    key = (dbg, reps)
    if key not in _PROG_CACHE:
        _PROG_CACHE[key] = _build_program(dbg, reps)
    return _PROG_CACHE[key]


def kernel(**inputs) -> np.ndarray:
    from concourse.bass_utils import run_bass_kernel_spmd
    nc = get_program(dbg=False)
    in_maps = make_in_maps(**inputs)
    res = run_bass_kernel_spmd(nc, in_maps, list(range(NCORES)))
    out = np.zeros((T, D), np.float32)
    for c in range(NCORES):
        out += res.results[c]["outp"].astype(np.float32)
    return out.reshape(B, T, D)


# revision 39
# speedup vs baseline: 8.9514x; 1.1766x over previous
"""Trainium2 Bass kernel for the PrimedGKA layer (gated linear attention with
Chebyshev query refinement), tensor-parallel over the 16 query heads across
8 NeuronCores (2 q-heads + their shared kv-head per core); per-core partial
out-projections are summed on the host.

Restructured vs the baseline to relieve the DVE and PE-sequencer bottlenecks:
feature-major gate projection transposed once via the DMA XBAR, one batched
cumsum matmul for all chunks' decay logs, decay masks via Activation-engine
exp with a per-partition -G_s bias (PE matmul supplies the G_t broadcast and
the -30000 causal clamp), the causal conv as 4 accumulating diagonal matmuls
on PE, raw-k score matmuls with 1/||k|| folded into the query/value sides,
head-stacked 128-partition pass-2 layout so the per-head recurrent-state
matmuls merge into block-diagonal single matmuls, and elementwise traffic
split across DVE / GpSimd / Act.

Self-contained: hardcodes all shapes from the problem spec.
"""
import numpy as np

B, T, D = 1, 1024, 1024
HQ, HKV, HK, HV = 16, 4, 64, 64
KW = 4
NCORES = 8
L = 128                 # chunk length
NCH = T // L            # 8 chunks
CHEB_DAMP = 0.25
EPS = 1e-6

_PROG_CACHE = {}


def _build_program(dbg=False, reps=1):
    import concourse.bacc as bacc
    import concourse.mybir as mybir
    from concourse.tile import TileContext

    dt = mybir.dt
    f32 = dt.float32
    f32r = dt.float32r
    f16 = dt.float16
    AF = mybir.ActivationFunctionType
    ALU = mybir.AluOpType
    X = mybir.AxisListType.X

    nc = bacc.Bacc("TRN2", target_bir_lowering=False, debug=False,
                   num_devices=NCORES)

    xT16 = nc.dram_tensor("xT16", [D, T], f16, kind="ExternalInput")
    wcat = nc.dram_tensor("wcat", [D, 256], f16, kind="ExternalInput")
    wg5 = nc.dram_tensor("wg5", [D, 5], f16, kind="ExternalInput")
    convd = nc.dram_tensor("convd", [128, 8 * 128], f16, kind="ExternalInput")
    wo = nc.dram_tensor("wo", [128, D], f16, kind="ExternalInput")
    alog = nc.dram_tensor("alog", [1, 2], f32, kind="ExternalInput")
    dtbbc = nc.dram_tensor("dtbbc", [128, 2], f32, kind="ExternalInput")
    um = nc.dram_tensor("um", [128, 128], f32, kind="ExternalInput")
    up16 = nc.dram_tensor("up16", [128, 128], f16, kind="ExternalInput")
    rhs30k = nc.dram_tensor("rhs30k", [128, 512], f16, kind="ExternalInput")
    iden = nc.dram_tensor("iden", [128, 128], f16, kind="ExternalInput")
    selall = nc.dram_tensor("selall", [16, 1024], f16, kind="ExternalInput")
    wcvq = nc.dram_tensor("wcvq", [128, 4], f32, kind="ExternalInput")
    outp = nc.dram_tensor("outp", [T, D], f16, kind="ExternalOutput")
    if dbg:
        dSq = nc.dram_tensor("dSq", [128, T], f16, kind="ExternalOutput")
        dSkv = nc.dram_tensor("dSkv", [128, T], f16, kind="ExternalOutput")
        dgtm = nc.dram_tensor("dgtm", [128, 64], f16, kind="ExternalOutput")
        dgt = nc.dram_tensor("dgt", [128, 2048], f16, kind="ExternalOutput")
        dgb = nc.dram_tensor("dgb", [128, 1024], f16, kind="ExternalOutput")
        dkvtm = nc.dram_tensor("dkvtm", [8, 128, 192], f16, kind="ExternalOutput")
        dhmk = nc.dram_tensor("dhmk", [8, 128, 128], f16, kind="ExternalOutput")
        dhmv = nc.dram_tensor("dhmv", [8, 128, 128], f16, kind="ExternalOutput")
        dqse = nc.dram_tensor("dqse", [128, 2048], f16, kind="ExternalOutput")
        dxce = nc.dram_tensor("dxce", [8, 128, 256], f16, kind="ExternalOutput")
        dsfin = nc.dram_tensor("dsfin", [128, 16], f32, kind="ExternalOutput")
        dinvk = nc.dram_tensor("dinvk", [128, 8], f32, kind="ExternalOutput")

    with TileContext(nc) as tc:
      import contextlib
      for _rep in range(reps):
        ctx = contextlib.ExitStack()
        with ctx:
            pers = ctx.enter_context(tc.tile_pool(name="pers", bufs=1))
            p_sm = ctx.enter_context(tc.tile_pool(name="p_sm", bufs=9))
            p_kv = ctx.enter_context(tc.tile_pool(name="p_kv", bufs=9))
            p_xq = ctx.enter_context(tc.tile_pool(name="p_xq", bufs=10))
            p_out = ctx.enter_context(tc.tile_pool(name="p_out", bufs=6))
            ps_all = ctx.enter_context(tc.tile_pool(name="ps_all", bufs=6, space="PSUM"))
            ps_oy = ctx.enter_context(tc.tile_pool(name="ps_oy", bufs=2, space="PSUM"))

            # ================= persistent loads =================
            wcat_sb = pers.tile([128, 8, 256], f16)
            nc.sync.dma_start(out=wcat_sb[:], in_=wcat[:].rearrange("(a p) c -> p a c", p=128))
            wg5_sb = pers.tile([128, 8, 5], f16)
            nc.sync.dma_start(out=wg5_sb[:], in_=wg5[:].rearrange("(a p) c -> p a c", p=128))
            convd_sb = pers.tile([128, 8, 128], f16)
            nc.scalar.dma_start(out=convd_sb[:], in_=convd[:].rearrange("p (a c) -> p a c", a=8))
            xt16_sb = pers.tile([128, 8, T], f16)
            for d in range(8):
                eng = nc.sync if d % 2 == 0 else nc.scalar
                eng.dma_start(out=xt16_sb[:, d, :], in_=xT16[d * 128:(d + 1) * 128, :])
            wo_sb = pers.tile([128, D], f16)
            nc.scalar.dma_start(out=wo_sb[:], in_=wo[:])
            alog_sb = pers.tile([1, 2], f32)
            nc.sync.dma_start(out=alog_sb[:], in_=alog[:])
            dtb_sb = pers.tile([128, 2], f32)
            nc.sync.dma_start(out=dtb_sb[:], in_=dtbbc[:])
            um_sb = pers.tile([128, 128], f32)
            nc.sync.dma_start(out=um_sb[:], in_=um[:])
            up16_sb = pers.tile([128, 128], f16)
            nc.scalar.dma_start(out=up16_sb[:], in_=up16[:])
            rhs30k_sb = pers.tile([128, 512], f16)
            nc.scalar.dma_start(out=rhs30k_sb[:], in_=rhs30k[:])
            iden16_sb = pers.tile([128, 128], f16)
            nc.sync.dma_start(out=iden16_sb[:], in_=iden[:])
            selall_sb = pers.tile([16, 1024], f16)
            nc.scalar.dma_start(out=selall_sb[:], in_=selall[:])
            wcvq_sb = pers.tile([128, 4], f32)
            nc.sync.dma_start(out=wcvq_sb[:], in_=wcvq[:])

            ones128h = pers.tile([1, 128], f16)
            nc.vector.memset(ones128h[:], 1.0)

            # persistent work tiles
            Rq = pers.tile([128, 3 + T], f16)     # padded raw q proj, fm
            Rkv = pers.tile([128, 3 + T], f16)    # padded raw k|v proj, fm
            nc.vector.memset(Rq[:, 0:3], 0.0)
            nc.vector.memset(Rkv[:, 0:3], 0.0)
            Sq = pers.tile([128, T], f16)         # silu(conv(q)), fm (h-stacked)
            Skv = pers.tile([128, T], f16)        # silu(conv(k|v)), fm
            kst = pers.tile([128, T], f16)        # [k ; k] duplicated rows
            gF = pers.tile([5, T], f16)           # feature-major gates
            g_tmT = pers.tile([128, NCH * 8], f16)    # time-major gates (c)(8)
            e_a = pers.tile([128, 16], f32)
            sp_a = pers.tile([128, 16], f32)
            g_all = pers.tile([128, 16], f32)
            e_ab = pers.tile([128, 24], f32)
            d_ab = pers.tile([128, 24], f32)
            ab_all = pers.tile([128, 24], f32)    # (c)(al0, al1, beta) sigmoids
            asq = pers.tile([128, 16], f32)       # alpha^2 (c)(h)
            negG = pers.tile([128, 16], f32)
            growall = pers.tile([16, 128], f32)
            growhi = pers.tile([16, 128], f16)
            growres = pers.tile([16, 128], f16)
            growrowh = pers.tile([1, 2048], f16)
            growrowr = pers.tile([1, 2048], f16)
            Cq = pers.tile([128, T], f16)
            gamlog16 = pers.tile([16, 128], f16)
            gt_all = pers.tile([128, 2048], f16)  # decay masks (c)(h)(t)
            gb_st = pers.tile([128, 1024], f16)   # gamma, head-stacked (c)(t)
            sqk_all = pers.tile([128, 512], f16)
            ssk = pers.tile([128, 8], f32)
            lnk = pers.tile([128, 8], f32)
            invk = pers.tile([128, 8], f32)
            sq_all = pers.tile([128, 1024], f16)  # rms squares (c)(h*64)
            sso = pers.tile([128, 16], f32)
            uvar = pers.tile([128, 16], f32)
            lno = pers.tile([128, 16], f32)
            sfac = pers.tile([128, 16], f32)
            sfin = pers.tile([128, 16], f32)
            era = pers.tile([1, 2], f16)
            negea = pers.tile([128, 2], f32)
            glf = pers.tile([128, 8], f32)        # gammaL per chunk, h-stacked
            b1em12 = pers.tile([128, 1], f32)
            nc.vector.memset(b1em12[:], 1e-12)
            bepsk = pers.tile([128, 1], f32)
            nc.vector.memset(bepsk[:], EPS * HK)

            # q in block-diagonal stacked layout: rows 0:64 head0 (cols c*256
            # .. +128), rows 64:128 head1 (cols c*256+128 .. +256), 0 elsewhere
            qse = pers.tile([128, 2048], f16)
            nc.vector.memset(qse[0:64, :].rearrange("p (c a t) -> p c a t", c=NCH, a=2)[:, :, 1, :], 0.0)
            nc.vector.memset(qse[64:128, :].rearrange("p (c a t) -> p c a t", c=NCH, a=2)[:, :, 0, :], 0.0)

            kvt = [pers.tile([128, 128], f16, name=f"kvt{c}") for c in range(NCH)]
            kvtm = [pers.tile([128, 192], f16, name=f"kvtm{c}") for c in range(NCH)]
            kw = [pers.tile([128, 128], f16, name=f"kw{c}") for c in range(NCH)]
            knegp = [pers.tile([128, 256], f16, name=f"knegp{c}") for c in range(NCH)]
            hmk = [pers.tile([128, 128], f16, name=f"hmk{c}") for c in range(NCH)]
            hmv = [pers.tile([128, 128], f16, name=f"hmv{c}") for c in range(NCH)]
            xceall = pers.tile([128, 2048], f16)
            xv0 = xceall[0:64, :].rearrange("p (c a t) -> p c a t", c=NCH, a=2)
            xv1 = xceall[64:128, :].rearrange("p (c a t) -> p c a t", c=NCH, a=2)
            nc.vector.memset(xv0[:, :, 1, :], 0.0)
            nc.vector.memset(xv1[:, :, 0, :], 0.0)
            for c in range(NCH):
                # zero-pad: knegp halves, hm block-diag off-blocks, xce quadrants
                nc.gpsimd.memset(knegp[c][:, 64:192], 0.0)
                if c > 0:
                    nc.gpsimd.memset(hmk[c][0:64, 64:128], 0.0)
                    nc.gpsimd.memset(hmk[c][64:128, 0:64], 0.0)
                    nc.gpsimd.memset(hmv[c][0:64, 64:128], 0.0)
                    nc.gpsimd.memset(hmv[c][64:128, 0:64], 0.0)


            # ============ front: projections / gates / conv, interleaved ====
            # ordering tuned so each engine queue receives work in expected
            # readiness order (in-order queues suffer head-of-line blocking)
            proj_ps = {}

            def proj_mms(ct, dlist):
                c0 = ct * 128
                if ct not in proj_ps:
                    proj_ps[ct] = [ps_all.tile([128, 512], f32, tag="ps",
                                               name=f"pj{ct}{th}") for th in range(2)]
                for d in dlist:
                    for th in range(2):
                        nc.tensor.matmul(proj_ps[ct][th][:], wcat_sb[:, d, c0:c0 + 128],
                                         xt16_sb[:, d, th * 512:(th + 1) * 512],
                                         start=(d == 0), stop=(d == 7))

            def proj_copies(ct, R):
                for th in range(2):
                    dst = R[:, 3 + th * 512: 3 + (th + 1) * 512]
                    nc.vector.tensor_copy(dst, proj_ps[ct][th][:])

            def conv_mms(tile_idx, R):
                cps = [ps_all.tile([128, 512], f32, tag="ps", name=f"cv{tile_idx}{th}")
                       for th in range(2)]
                for i in range(KW):
                    for th in range(2):
                        nc.tensor.matmul(cps[th][:],
                                         convd_sb[:, tile_idx * 4 + i, :],
                                         R[:, i + th * 512: i + th * 512 + 512],
                                         start=(i == 0), stop=(i == KW - 1))
                return cps

            # -- kv path first: projection, conv, silu, transposes --
            proj_mms(1, range(0, 8))
            proj_copies(1, Rkv)
            cps_kv = conv_mms(1, Rkv)
            for th in range(2):
                nc.scalar.activation(Skv[:, th * 512:(th + 1) * 512], cps_kv[th][:], AF.Silu)
            nc.sync.dma_start(out=kst[0:64, :], in_=Skv[0:64, :])
            nc.scalar.dma_start(out=kst[64:128, :], in_=Skv[0:64, :])
            for c in range(NCH):
                eng = nc.sync if c % 2 == 0 else nc.scalar
                eng.dma_start(out=kvt[c][:], in_=Skv[:, c * L:(c + 1) * L],
                              transpose=True)

            # -- gate projection + transposes --
            pg = [ps_all.tile([5, 512], f32, tag="ps", name=f"pg{th}") for th in range(2)]
            for d in range(8):
                for th in range(2):
                    nc.tensor.matmul(pg[th][:], wg5_sb[:, d, :],
                                     xt16_sb[:, d, th * 512:(th + 1) * 512],
                                     start=(d == 0), stop=(d == 7))
            for th in range(2):
                nc.scalar.copy(gF[:, th * 512:(th + 1) * 512], pg[th][:])
            ps_gt = ps_all.tile([128, 64], f16, tag="ps", name="psgt")
            for c in range(NCH):
                nc.tensor.transpose(ps_gt[:, c * 8:c * 8 + 5],
                                    gF[:, c * 128:(c + 1) * 128],
                                    iden16_sb[0:5, 0:5])
            nc.vector.tensor_copy(g_tmT[:], ps_gt[:])

            # -- gate chain (Act/DVE) --
            gv = g_tmT[:].rearrange("p (c r) -> p c r", r=8)
            eav = e_a[:].rearrange("p (c h) -> p c h", h=2)
            for h in range(2):
                nc.scalar.activation(eav[:, :, h], gv[:, :, h], AF.Exp,
                                     bias=dtb_sb[:, h:h + 1])
            nc.scalar.activation(sp_a[:], e_a[:], AF.Ln, bias=1.0)
            nc.scalar.activation(era[:], alog_sb[:], AF.Exp)
            ps_bc = ps_all.tile([128, 128], f32, tag="ps", name="psbc")
            nc.tensor.matmul(ps_bc[:, 0:2], ones128h[:], era[:], start=True, stop=True)
            nc.vector.tensor_scalar(negea[:], ps_bc[:, 0:2], -1.0, None, ALU.mult)
            spv = sp_a[:].rearrange("p (c h) -> p c h", h=2)
            gav = g_all[:].rearrange("p (c h) -> p c h", h=2)
            for h in range(2):
                nc.vector.tensor_scalar(gav[:, :, h], spv[:, :, h],
                                        negea[:, h:h + 1], None, ALU.mult)
            nc.scalar.activation(e_ab[:].rearrange("p (c r) -> p c r", r=3),
                                 gv[:, :, 2:5], AF.Exp, scale=-1.0)
            nc.vector.tensor_scalar(d_ab[:], e_ab[:], 1.0, None, ALU.add)
            nc.vector.reciprocal(ab_all[:], d_ab[:])
            nc.scalar.activation(asq[:].rearrange("p (c h) -> p c h", h=2),
                                 ab_all[:].rearrange("p (c r) -> p c r", r=3)[:, :, 0:2],
                                 AF.Square)

            # -- k-norm squares + per-pair invk (Act ops early in queue) --
            for c in range(NCH):
                nc.vector.tensor_tensor(sqk_all[:, c * 64:(c + 1) * 64],
                                        kvt[c][:, 0:64], kvt[c][:, 0:64], ALU.mult)
                if c % 2 == 1:
                    nc.vector.tensor_reduce(
                        ssk[:, c - 1:c + 1],
                        sqk_all[:, (c - 1) * 64:(c + 1) * 64]
                        .rearrange("p (c k) -> p c k", c=2), X, ALU.add)
                    nc.scalar.activation(lnk[:, c - 1:c + 1], ssk[:, c - 1:c + 1],
                                         AF.Ln, bias=b1em12[:])
                    nc.scalar.activation(invk[:, c - 1:c + 1], lnk[:, c - 1:c + 1],
                                         AF.Exp, scale=-0.5)

            # -- q path (cumsum + mask matmuls interleaved into the PE stream) --
            proj_mms(0, range(0, 4))
            psG = ps_all.tile([128, 16], f32, tag="ps", name="psG")
            nc.tensor.matmul(psG[:], um_sb[:], g_all[:], start=True, stop=True)
            nc.vector.tensor_scalar(negG[:], psG[:], -1.0, None, ALU.mult)
            psGr = ps_all.tile([16, 128], f32, tag="ps", name="psGr")
            nc.tensor.matmul(psGr[:], g_all[:], um_sb[:], start=True, stop=True)
            nc.vector.tensor_copy(growall[:], psGr[:])
            nc.vector.tensor_copy(growhi[:], psGr[:])
            nc.vector.tensor_tensor(growres[:], growall[:], growhi[:], ALU.subtract)
            nc.sync.dma_start(out=growrowh[:], in_=growhi[:])
            nc.scalar.dma_start(out=growrowr[:], in_=growres[:])
            nc.vector.tensor_copy(gamlog16[:], psGr[:])

            proj_mms(0, range(4, 8))
            # gamma broadcast (log-domain selector matmul, exp)
            for b in range(2):
                psgb = ps_all.tile([128, 512], f32, tag="ps", name=f"psgb{b}")
                for k in range(4):
                    c = b * 4 + k
                    nc.tensor.matmul(psgb[:, k * 128:(k + 1) * 128],
                                     selall_sb[:, c * 128:(c + 1) * 128],
                                     gamlog16[:], start=True, stop=True)
                for k in range(2):
                    nc.scalar.activation(gb_st[:, b * 512 + k * 256: b * 512 + (k + 1) * 256],
                                         psgb[:, k * 256:(k + 1) * 256], AF.Exp)
            nc.vector.tensor_copy(
                glf[:], gb_st[:].rearrange("p (c t) -> p c t", c=NCH)[:, :, L - 1])

            # decay masks
            for b in range(4):
                psGB = ps_all.tile([128, 512], f32, tag="ps", name=f"psGB{b}")
                nc.tensor.matmul(psGB[:], ones128h[:], growrowh[:, b * 512:(b + 1) * 512],
                                 start=True, stop=False)
                nc.tensor.matmul(psGB[:], ones128h[:], growrowr[:, b * 512:(b + 1) * 512],
                                 start=False, stop=False)
                nc.tensor.matmul(psGB[:], up16_sb[:], rhs30k_sb[:],
                                 start=False, stop=True)
                for k in range(4):
                    r = b * 4 + k
                    nc.scalar.activation(gt_all[:, r * 128:(r + 1) * 128],
                                         psGB[:, k * 128:(k + 1) * 128], AF.Exp,
                                         bias=negG[:, r:r + 1])

            proj_copies(0, Rq)
            cps_q = conv_mms(0, Rq)
            for th in range(2):
                nc.scalar.activation(Sq[:, th * 512:(th + 1) * 512], cps_q[th][:], AF.Silu)

            # ============ kvtm + kw + state chain (per chunk) ======
            hm_prev = None
            for c in range(NCH):
                km = kvtm[c]
                nc.vector.tensor_scalar(km[:, 0:64], kvt[c][:, 0:64],
                                        invk[:, c:c + 1], None, ALU.mult)
                nc.vector.tensor_scalar(km[:, 64:128], kvt[c][:, 64:128],
                                        ab_all[:, c * 3 + 2:c * 3 + 3], None, ALU.mult)
                nc.vector.tensor_scalar(km[:, 128:192], km[:, 64:128],
                                        invk[:, c:c + 1], None, ALU.mult)
                nc.gpsimd.tensor_scalar(knegp[c][:, 0:64], km[:, 0:64],
                                        invk[:, c:c + 1], -CHEB_DAMP, ALU.mult, ALU.mult)
                nc.gpsimd.tensor_copy(knegp[c][:, 192:256], knegp[c][:, 0:64])
                for h in range(2):
                    nc.vector.tensor_tensor(
                        kw[c][:, h * 64:(h + 1) * 64], km[:, 0:64],
                        gt_all[:, c * 256 + h * 128 + L - 1: c * 256 + h * 128 + L]
                        .broadcast_to([128, 64]), ALU.mult)
                ps_hm = ps_all.tile([128, 128], f32, tag="ps", name=f"pshm{c}")
                nc.tensor.matmul(ps_hm[:], kw[c][:], kvtm[c][:, 0:128],
                                 start=True, stop=(c == 0))
                if c > 0:
                    dg = p_sm.tile([128, 128], f16, tag="diag", name=f"diag{c}")
                    nc.vector.tensor_scalar(dg[:], iden16_sb[:],
                                            glf[:, c:c + 1], None, ALU.mult)
                    nc.tensor.matmul(ps_hm[:], dg[:], hm_prev[:],
                                     start=False, stop=True)
                if c + 1 < NCH:
                    hm = p_kv.tile([128, 128], f16, tag="hm", name=f"hm{c}")
                    nc.vector.tensor_copy(hm[:], ps_hm[:])
                    hm_prev = hm
                    cn = c + 1
                    nc.vector.tensor_scalar(hmk[cn][0:64, 0:64], ps_hm[0:64, 0:64],
                                            -CHEB_DAMP, None, ALU.mult)
                    nc.vector.tensor_scalar(hmk[cn][64:128, 64:128], ps_hm[64:128, 0:64],
                                            -CHEB_DAMP, None, ALU.mult)
                    nc.vector.tensor_copy(hmv[cn][0:64, 0:64], ps_hm[0:64, 64:128])
                    nc.vector.tensor_copy(hmv[cn][64:128, 64:128], ps_hm[64:128, 64:128])

            if dbg and _rep == 0:
                nc.sync.dma_start(out=dSq[:], in_=Sq[:])
                nc.sync.dma_start(out=dSkv[:], in_=Skv[:])
                nc.sync.dma_start(out=dgtm[:], in_=g_tmT[:])
                nc.sync.dma_start(out=dgt[:], in_=gt_all[:])
                nc.sync.dma_start(out=dgb[:], in_=gb_st[:])
                nc.sync.dma_start(out=dinvk[:], in_=invk[:])
                for c in range(NCH):
                    nc.sync.dma_start(out=dkvtm[c], in_=kvtm[c][:])
                    if c > 0:
                        nc.sync.dma_start(out=dhmk[c], in_=hmk[c][:])
                        nc.sync.dma_start(out=dhmv[c], in_=hmv[c][:])

            # stacked q into block-diag quadrants (feeds pass-2 scores)
            qv = qse[:].rearrange("p (c a t) -> p c a t", c=NCH, a=2)
            nc.vector.tensor_copy(
                qv[0:64, :, 0, :],
                Sq[0:64, :].rearrange("p (c t) -> p c t", c=NCH))
            nc.vector.tensor_copy(
                qv[64:128, :, 1, :],
                Sq[64:128, :].rearrange("p (c t) -> p c t", c=NCH))

            # ============ pass 2: operator applications ============
            grp = list(range(NCH))
            xcur = {ci: qse[:, ci * 256:(ci + 1) * 256] for ci in grp}
            asb = {}
            ps_o = {}
            for it in range(3):
                ps_p = {}
                pspb_by = {}
                for ci in grp:
                    if ci % 2 == 0:
                        pspb = ps_all.tile([128, 512], f32, tag="ps", name=f"pspb{ci}")
                    pspb_by[ci] = pspb
                    ps_p[ci] = pspb[:, (ci % 2) * 256:(ci % 2 + 1) * 256]
                    nc.tensor.matmul(ps_p[ci], kst[:, ci * L:(ci + 1) * L],
                                     xcur[ci], start=True, stop=True)
                for ci in grp:
                    if ci % 2 == 1:
                        scr = p_kv.tile([128, 512], f16, tag="scr", name=f"scr{ci}")
                        nc.scalar.copy(scr[:], pspb_by[ci][:])
                        a2 = p_kv.tile([128, 512], f16, tag="a", name=f"asb{ci}")
                        nc.vector.tensor_tensor(
                            a2[:], scr[:],
                            gt_all[:, (ci - 1) * 256:(ci + 1) * 256], ALU.mult)
                        asb[ci - 1] = a2[:, 0:256]
                        asb[ci] = a2[:, 256:512]
                xgs = {}
                for ci in grp:
                    if ci > 0:
                        xs = p_xq.tile([128, 128], f16, tag="xgs", name=f"xgs{ci}")
                        nc.vector.tensor_tensor(
                            xs[0:64, :], xcur[ci][0:64, 0:128],
                            gb_st[0:64, ci * 128:(ci + 1) * 128], ALU.mult)
                        nc.gpsimd.tensor_tensor(
                            xs[64:128, :], xcur[ci][64:128, 128:256],
                            gb_st[64:128, ci * 128:(ci + 1) * 128], ALU.mult)
                        xgs[ci] = xs
                if it < 2:
                    ps_y = {}
                    for ci in grp:
                        if ci % 4 == 0:
                            psyb = ps_oy.tile([128, 512], f32, tag="psy", name=f"psyb{it}{ci}")
                        ps_y[ci] = psyb[:, (ci % 4) * 128:(ci % 4 + 1) * 128]
                        nc.tensor.matmul(ps_y[ci], knegp[ci][:, 0:128], asb[ci][:, 0:128],
                                         start=True, stop=False)
                        nc.tensor.matmul(ps_y[ci], knegp[ci][:, 128:256], asb[ci][:, 128:256],
                                         start=False, stop=(ci == 0))
                        if ci > 0:
                            nc.tensor.matmul(ps_y[ci], hmk[ci][:], xgs[ci][:],
                                             start=False, stop=True)
                        if ci % 4 == 3:
                            p0 = ci - 3
                            scry = p_xq.tile([128, 512], f16, tag="scry", name=f"scry{it}{ci}")
                            nc.scalar.copy(scry[:], psyb[:])
                            for cj in range(p0, p0 + 4):
                                rg = (cj % 4) * 128
                                nc.vector.tensor_tensor(
                                    xceall[0:64, cj * 256:cj * 256 + 128],
                                    scry[0:64, rg:rg + 128],
                                    qse[0:64, cj * 256:cj * 256 + 128], ALU.add)
                                nc.vector.tensor_tensor(
                                    xceall[64:128, cj * 256 + 128:(cj + 1) * 256],
                                    scry[64:128, rg:rg + 128],
                                    qse[64:128, cj * 256 + 128:(cj + 1) * 256], ALU.add)
                                xcur[cj] = xceall[:, cj * 256:(cj + 1) * 256]
                else:
                    for ci in grp:
                        if ci % 4 == 0:
                            psob = ps_oy.tile([128, 512], f32, tag="psy", name=f"psob{ci}")
                        ps_o[ci] = psob[:, (ci % 4) * 128:(ci % 4 + 1) * 128]
                        for h in range(2):
                            nc.tensor.matmul(ps_o[ci][:, h * 64:(h + 1) * 64],
                                             asb[ci][:, h * 128:(h + 1) * 128],
                                             kvtm[ci][:, 128:192],
                                             start=True, stop=(ci == 0))
                            if ci > 0:
                                nc.tensor.matmul(ps_o[ci][:, h * 64:(h + 1) * 64],
                                                 xgs[ci][:],
                                                 hmv[ci][:, h * 64:(h + 1) * 64],
                                                 start=False, stop=True)

            # ---- rmsnorm + alpha gate + out projection, in two halves ----
            out_rr = 0
            for half in range(2):
                hgrp = list(range(half * 4, half * 4 + 4))
                for ci in hgrp:
                    nc.scalar.activation(sq_all[:, ci * 128:(ci + 1) * 128],
                                         ps_o[ci], AF.Square)
                g0 = half * 4
                sl8 = slice(g0 * 2, g0 * 2 + 8)
                nc.vector.tensor_reduce(
                    sso[:, sl8],
                    sq_all[:, g0 * 128:(g0 + 4) * 128]
                    .rearrange("p (r k) -> p r k", k=64), X, ALU.add)
                nc.vector.tensor_tensor(uvar[:, sl8], sso[:, sl8], asq[:, sl8], ALU.mult)
                nc.scalar.activation(lno[:, sl8], uvar[:, sl8], AF.Ln,
                                     bias=bepsk[:], scale=1.0 / HV)
                nc.scalar.activation(sfac[:, sl8], lno[:, sl8], AF.Exp, scale=-0.5)
                nc.vector.tensor_tensor(
                    sfin[:].rearrange("p (c h) -> p c h", h=2)[:, g0:g0 + 4, :],
                    sfac[:].rearrange("p (c h) -> p c h", h=2)[:, g0:g0 + 4, :],
                    ab_all[:].rearrange("p (c r) -> p c r", r=3)[:, g0:g0 + 4, 0:2],
                    ALU.mult)
                for ci in hgrp:
                    o = p_out.tile([128, 128], f16, tag="on", name=f"on{ci}")
                    nc.vector.tensor_tensor(
                        o[:].rearrange("p (h v) -> p h v", h=2), 
                        ps_o[ci][:].rearrange("p (h v) -> p h v", h=2),
                        sfin[:, ci * 2:ci * 2 + 2].unsqueeze(2).broadcast_to([128, 2, 64]),
                        ALU.mult)
                    ps_of = ps_all.tile([128, 128], f16, tag="ps", name=f"psof{ci}")
                    nc.tensor.transpose(ps_of[:], o[:], iden16_sb[:])
                    f = p_out.tile([128, 128], f16, tag="ofm", name=f"ofm{ci}")
                    nc.vector.tensor_copy(f[:], ps_of[:])
                    out_sb = p_out.tile([128, D], f16, tag="outsb")
                    for nh in range(2):
                        ps_out = ps_all.tile([128, 512], f32, tag="ps", name=f"pso2{ci}{nh}")
                        nc.tensor.matmul(ps_out[:], f[:],
                                         wo_sb[:, nh * 512:(nh + 1) * 512],
                                         start=True, stop=True)
                        dst = out_sb[:, nh * 512:(nh + 1) * 512]
                        nc.scalar.copy(dst, ps_out[:])
                    nc.sync.dma_start(out=outp[ci * L:(ci + 1) * L, 0:512],
                                      in_=out_sb[:, 0:512])
                    nc.scalar.dma_start(out=outp[ci * L:(ci + 1) * L, 512:1024],
                                        in_=out_sb[:, 512:1024])

    # Activation-table placement: map Exp/Ln/Square/Copy into the combined
    # natural_log_exp table (hoisted first) so only Silu forces a switch.
    import concourse.bacc as bacc_mod
    from concourse.hw_specs import get_activation_tables as _gat
    orig_tables = _gat(nc.m.arch)
    orig_names = list(orig_tables.keys())
    pref = "natural_log_exp_and_others"
    reordered = {pref: orig_tables[pref],
                 **{k: v for k, v in orig_tables.items() if k != pref}}
    pnames = list(reordered.keys())
    bacc_mod.get_activation_tables = lambda arch: reordered
    try:
        nc.compile()
    finally:
        bacc_mod.get_activation_tables = _gat
    for b in nc.main_func.blocks:
        for i in b.instructions:
            if isinstance(i, mybir.InstLoadActFuncSet):
                i.act_func_set_id = orig_names.index(pnames[i.act_func_set_id])
    return nc


def _prep_core_inputs(c, x, Wq, Wk, Wv, Wconv, Wa, Walpha, Wb, A_log, dt_bias,
                      norm_w, Wo, xT16, iden16, um32, up16, rhs30k, selall):
    f32, f16 = np.float32, np.float16
    h0, h1, hk = 2 * c, 2 * c + 1, c // 2
    wcat = np.hstack([
        Wq[:, h0 * HK:(h0 + 1) * HK], Wq[:, h1 * HK:(h1 + 1) * HK],
        Wk[:, hk * HK:(hk + 1) * HK], Wv[:, hk * HV:(hk + 1) * HV],
    ]).astype(f16)
    wg5 = np.hstack([
        Wa[:, h0:h0 + 1], Wa[:, h1:h1 + 1],
        Walpha[:, h0:h0 + 1], Walpha[:, h1:h1 + 1],
        Wb[:, hk:hk + 1],
    ]).astype(f16)
    qoff, koff, voff = 0, HQ * HK, HQ * HK + HKV * HK
    wcv = np.vstack([
        Wconv[qoff + h0 * HK: qoff + (h0 + 1) * HK],
        Wconv[qoff + h1 * HK: qoff + (h1 + 1) * HK],
        Wconv[koff + hk * HK: koff + (hk + 1) * HK],
        Wconv[voff + hk * HV: voff + (hk + 1) * HV],
    ]).astype(f32)  # [256, 4]: rows = [q(128) | k(64) | v(64)]
    convd = np.zeros((128, 8, 128), f32)
    for tile in range(2):
        ch = wcv[tile * 128:(tile + 1) * 128]   # [128, KW]
        for tap in range(KW):
            np.fill_diagonal(convd[:, tile * 4 + tap, :], ch[:, tap])
    convd = np.ascontiguousarray(convd.reshape(128, 8 * 128)).astype(f16)
    wcvq = np.ascontiguousarray(wcv[0:128]).astype(f32)
    wo_scale = np.tile(np.asarray(norm_w, f32), HQ)
    Wo_s = np.asarray(Wo, f32) * wo_scale[:, None]
    wo = np.ascontiguousarray(
        np.vstack([Wo_s[h0 * HV:(h0 + 1) * HV], Wo_s[h1 * HV:(h1 + 1) * HV]])).astype(f16)
    alog = np.asarray(A_log, f32)[[h0, h1]].reshape(1, 2).copy()
    dtbbc = np.tile(np.asarray(dt_bias, f32)[[h0, h1]].reshape(1, 2), (128, 1)).copy()
    return dict(xT16=xT16, wcat=np.ascontiguousarray(wcat), wg5=wg5,
                convd=convd, wcvq=wcvq, wo=wo, alog=alog, dtbbc=dtbbc,
                um=um32, up16=up16, rhs30k=rhs30k, iden=iden16, selall=selall)


def make_in_maps(x, Wq, Wk, Wv, Wconv, Wa, Walpha, Wb, A_log, dt_bias, norm_w, Wo):
    f32, f16 = np.float32, np.float16
    x2 = np.asarray(x, f32).reshape(T, D)
    xT16 = np.ascontiguousarray(x2.T).astype(f16)
    iden16 = np.eye(128, dtype=f16)
    um32 = np.ascontiguousarray(np.triu(np.ones((128, 128), f32)))
    up16 = np.ascontiguousarray(np.triu(np.ones((128, 128), f16), 1))
    rhs30k = np.ascontiguousarray(
        np.hstack([-30000.0 * np.eye(128)] * 4)).astype(f16)
    selall = np.zeros((16, 8, 128), f16)
    for c in range(8):
        selall[c * 2, c, 0:64] = 1.0
        selall[c * 2 + 1, c, 64:128] = 1.0
    selall = np.ascontiguousarray(selall.reshape(16, 1024))
    args = (x, np.asarray(Wq, f32), np.asarray(Wk, f32), np.asarray(Wv, f32),
            np.asarray(Wconv, f32), np.asarray(Wa, f32), np.asarray(Walpha, f32),
            np.asarray(Wb, f32), A_log, dt_bias, norm_w, Wo)
    return [_prep_core_inputs(c, *args, xT16=xT16, iden16=iden16, um32=um32,
                              up16=up16, rhs30k=rhs30k, selall=selall)
            for c in range(NCORES)]


def get_program(dbg=False, reps=1):
    key = (dbg, reps)
    if key not in _PROG_CACHE:
        _PROG_CACHE[key] = _build_program(dbg, reps)
    return _PROG_CACHE[key]


def kernel(**inputs) -> np.ndarray:
    from concourse.bass_utils import run_bass_kernel_spmd
    nc = get_program(dbg=False)
    in_maps = make_in_maps(**inputs)
    res = run_bass_kernel_spmd(nc, in_maps, list(range(NCORES)))
    out = np.zeros((T, D), np.float32)
    for c in range(NCORES):
        out += res.results[c]["outp"].astype(np.float32)
    return out.reshape(B, T, D)


# revision 44
# speedup vs baseline: 11.7803x; 1.3160x over previous
"""Trainium2 Bass kernel for the PrimedGKA layer (gated linear attention with
Chebyshev query refinement), tensor-parallel over the 16 query heads across
8 NeuronCores (2 q-heads + their shared kv-head per core); per-core partial
out-projections are summed on the host.

Restructured vs the baseline to relieve the DVE and PE-sequencer bottlenecks:
feature-major gate projection transposed once via the DMA XBAR, one batched
cumsum matmul for all chunks' decay logs, decay masks via Activation-engine
exp with a per-partition -G_s bias (PE matmul supplies the G_t broadcast and
the -30000 causal clamp), the causal conv as 4 accumulating diagonal matmuls
on PE, raw-k score matmuls with 1/||k|| folded into the query/value sides,
head-stacked 128-partition pass-2 layout so the per-head recurrent-state
matmuls merge into block-diagonal single matmuls, and elementwise traffic
split across DVE / GpSimd / Act.

Self-contained: hardcodes all shapes from the problem spec.
"""
import numpy as np

B, T, D = 1, 1024, 1024
HQ, HKV, HK, HV = 16, 4, 64, 64
KW = 4
NCORES = 8
L = 128                 # chunk length
NCH = T // L            # 8 chunks
CHEB_DAMP = 0.25
EPS = 1e-6

_PROG_CACHE = {}


def _build_program(dbg=False, reps=1):
    import concourse.bacc as bacc
    import concourse.mybir as mybir
    from concourse.tile import TileContext

    dt = mybir.dt
    f32 = dt.float32
    f32r = dt.float32r
    f16 = dt.float16
    AF = mybir.ActivationFunctionType
    ALU = mybir.AluOpType
    X = mybir.AxisListType.X

    nc = bacc.Bacc("TRN2", target_bir_lowering=False, debug=False,
                   num_devices=NCORES)

    xT16 = nc.dram_tensor("xT16", [D, T], f16, kind="ExternalInput")
    wcat = nc.dram_tensor("wcat", [D, 256], f16, kind="ExternalInput")
    wg5 = nc.dram_tensor("wg5", [D, 5], f16, kind="ExternalInput")
    convd = nc.dram_tensor("convd", [128, 8 * 128], f16, kind="ExternalInput")
    wo = nc.dram_tensor("wo", [128, D], f16, kind="ExternalInput")
    alog = nc.dram_tensor("alog", [1, 2], f32, kind="ExternalInput")
    dtbbc = nc.dram_tensor("dtbbc", [128, 2], f32, kind="ExternalInput")
    um = nc.dram_tensor("um", [128, 128], f32, kind="ExternalInput")
    up16 = nc.dram_tensor("up16", [128, 128], f16, kind="ExternalInput")
    rhs30k = nc.dram_tensor("rhs30k", [128, 512], f16, kind="ExternalInput")
    iden = nc.dram_tensor("iden", [128, 128], f16, kind="ExternalInput")
    selall = nc.dram_tensor("selall", [16, 1024], f16, kind="ExternalInput")
    wcvq = nc.dram_tensor("wcvq", [128, 4], f32, kind="ExternalInput")
    outp = nc.dram_tensor("outp", [T, D], f16, kind="ExternalOutput")
    if dbg:
        dSq = nc.dram_tensor("dSq", [128, T], f16, kind="ExternalOutput")
        dSkv = nc.dram_tensor("dSkv", [128, T], f16, kind="ExternalOutput")
        dgtm = nc.dram_tensor("dgtm", [128, 64], f16, kind="ExternalOutput")
        dgt = nc.dram_tensor("dgt", [128, 2048], f16, kind="ExternalOutput")
        dgb = nc.dram_tensor("dgb", [128, 1024], f16, kind="ExternalOutput")
        dkvtm = nc.dram_tensor("dkvtm", [8, 128, 192], f16, kind="ExternalOutput")
        dhmk = nc.dram_tensor("dhmk", [8, 128, 128], f16, kind="ExternalOutput")
        dhmv = nc.dram_tensor("dhmv", [8, 128, 128], f16, kind="ExternalOutput")
        dqse = nc.dram_tensor("dqse", [128, 2048], f16, kind="ExternalOutput")
        dxce = nc.dram_tensor("dxce", [8, 128, 256], f16, kind="ExternalOutput")
        dsfin = nc.dram_tensor("dsfin", [128, 16], f32, kind="ExternalOutput")
        dinvk = nc.dram_tensor("dinvk", [128, 8], f32, kind="ExternalOutput")

    with TileContext(nc) as tc:
      import contextlib
      for _rep in range(reps):
        ctx = contextlib.ExitStack()
        with ctx:
            pers = ctx.enter_context(tc.tile_pool(name="pers", bufs=1))
            p_sm = ctx.enter_context(tc.tile_pool(name="p_sm", bufs=9))
            p_kv = ctx.enter_context(tc.tile_pool(name="p_kv", bufs=9))
            p_xq = ctx.enter_context(tc.tile_pool(name="p_xq", bufs=10))
            p_out = ctx.enter_context(tc.tile_pool(name="p_out", bufs=6))
            ps_all = ctx.enter_context(tc.tile_pool(name="ps_all", bufs=6, space="PSUM"))
            ps_oy = ctx.enter_context(tc.tile_pool(name="ps_oy", bufs=2, space="PSUM"))

            # ================= persistent loads =================
            wcat_sb = pers.tile([128, 8, 256], f16)
            nc.sync.dma_start(out=wcat_sb[:], in_=wcat[:].rearrange("(a p) c -> p a c", p=128))
            wg5_sb = pers.tile([128, 8, 5], f16)
            nc.sync.dma_start(out=wg5_sb[:], in_=wg5[:].rearrange("(a p) c -> p a c", p=128))
            convd_sb = pers.tile([128, 8, 128], f16)
            nc.scalar.dma_start(out=convd_sb[:], in_=convd[:].rearrange("p (a c) -> p a c", a=8))
            xt16_sb = pers.tile([128, 8, T], f16)
            for d in range(8):
                eng = nc.sync if d % 2 == 0 else nc.scalar
                eng.dma_start(out=xt16_sb[:, d, :], in_=xT16[d * 128:(d + 1) * 128, :])
            wo_sb = pers.tile([128, D], f16)
            nc.scalar.dma_start(out=wo_sb[:], in_=wo[:])
            alog_sb = pers.tile([1, 2], f32)
            nc.sync.dma_start(out=alog_sb[:], in_=alog[:])
            dtb_sb = pers.tile([128, 2], f32)
            nc.sync.dma_start(out=dtb_sb[:], in_=dtbbc[:])
            um_sb = pers.tile([128, 128], f32)
            nc.sync.dma_start(out=um_sb[:], in_=um[:])
            up16_sb = pers.tile([128, 128], f16)
            nc.scalar.dma_start(out=up16_sb[:], in_=up16[:])
            rhs30k_sb = pers.tile([128, 512], f16)
            nc.scalar.dma_start(out=rhs30k_sb[:], in_=rhs30k[:])
            iden16_sb = pers.tile([128, 128], f16)
            nc.sync.dma_start(out=iden16_sb[:], in_=iden[:])
            selall_sb = pers.tile([16, 1024], f16)
            nc.scalar.dma_start(out=selall_sb[:], in_=selall[:])
            wcvq_sb = pers.tile([128, 4], f32)
            nc.sync.dma_start(out=wcvq_sb[:], in_=wcvq[:])

            ones128h = pers.tile([1, 128], f16)
            nc.vector.memset(ones128h[:], 1.0)

            # persistent work tiles
            Rq = pers.tile([128, 3 + T], f16)     # padded raw q proj, fm
            Rkv = pers.tile([128, 3 + T], f16)    # padded raw k|v proj, fm
            nc.vector.memset(Rq[:, 0:3], 0.0)
            nc.vector.memset(Rkv[:, 0:3], 0.0)
            Sq = pers.tile([128, T], f16)         # silu(conv(q)), fm (h-stacked)
            Skv = pers.tile([128, T], f16)        # silu(conv(k|v)), fm
            kst = pers.tile([128, T], f16)        # [k ; k] duplicated rows
            gF = pers.tile([5, T], f16)           # feature-major gates
            g_tmT = pers.tile([128, NCH * 8], f16)    # time-major gates (c)(8)
            e_a = pers.tile([128, 16], f32)
            sp_a = pers.tile([128, 16], f32)
            g_all = pers.tile([128, 16], f32)
            e_ab = pers.tile([128, 24], f32)
            d_ab = pers.tile([128, 24], f32)
            ab_all = pers.tile([128, 24], f32)    # (c)(al0, al1, beta) sigmoids
            asq = pers.tile([128, 16], f32)       # alpha^2 (c)(h)
            negG = pers.tile([128, 16], f32)
            growall = pers.tile([16, 128], f32)
            growhi = pers.tile([16, 128], f16)
            growres = pers.tile([16, 128], f16)
            growrowh = pers.tile([1, 2048], f16)
            growrowr = pers.tile([1, 2048], f16)
            Cq = pers.tile([128, T], f16)
            gamlog16 = pers.tile([16, 128], f16)
            gt_all = pers.tile([128, 2048], f16)  # decay masks (c)(h)(t)
            gb_st = pers.tile([128, 1024], f16)   # gamma, head-stacked (c)(t)
            sqk_all = pers.tile([128, 512], f16)
            ssk = pers.tile([128, 8], f32)
            lnk = pers.tile([128, 8], f32)
            invk = pers.tile([128, 8], f32)
            sq_all = pers.tile([128, 1024], f16)  # rms squares (c)(h*64)
            sso = pers.tile([128, 16], f32)
            uvar = pers.tile([128, 16], f32)
            lno = pers.tile([128, 16], f32)
            sfac = pers.tile([128, 16], f32)
            sfin = pers.tile([128, 16], f32)
            era = pers.tile([1, 2], f16)
            negea = pers.tile([128, 2], f32)
            glf = pers.tile([128, 8], f32)        # gammaL per chunk, h-stacked
            b1em12 = pers.tile([128, 1], f32)
            nc.vector.memset(b1em12[:], 1e-12)
            bepsk = pers.tile([128, 1], f32)
            nc.vector.memset(bepsk[:], EPS * HK)

            # q in block-diagonal stacked layout: rows 0:64 head0 (cols c*256
            # .. +128), rows 64:128 head1 (cols c*256+128 .. +256), 0 elsewhere
            qse = pers.tile([128, 2048], f16)
            nc.vector.memset(qse[0:64, :].rearrange("p (c a t) -> p c a t", c=NCH, a=2)[:, :, 1, :], 0.0)
            nc.vector.memset(qse[64:128, :].rearrange("p (c a t) -> p c a t", c=NCH, a=2)[:, :, 0, :], 0.0)

            kvt = [pers.tile([128, 128], f16, name=f"kvt{c}") for c in range(NCH)]
            kvtm = [pers.tile([128, 192], f16, name=f"kvtm{c}") for c in range(NCH)]
            kw = [pers.tile([128, 128], f16, name=f"kw{c}") for c in range(NCH)]
            knegp = [pers.tile([128, 256], f16, name=f"knegp{c}") for c in range(NCH)]
            hmk = [pers.tile([128, 128], f16, name=f"hmk{c}") for c in range(NCH)]
            hmv = [pers.tile([128, 128], f16, name=f"hmv{c}") for c in range(NCH)]
            xceall = pers.tile([128, 2048], f16)
            xv0 = xceall[0:64, :].rearrange("p (c a t) -> p c a t", c=NCH, a=2)
            xv1 = xceall[64:128, :].rearrange("p (c a t) -> p c a t", c=NCH, a=2)
            nc.vector.memset(xv0[:, :, 1, :], 0.0)
            nc.vector.memset(xv1[:, :, 0, :], 0.0)
            for c in range(NCH):
                # zero-pad: knegp halves, hm block-diag off-blocks, xce quadrants
                nc.gpsimd.memset(knegp[c][:, 64:192], 0.0)
                if c > 0:
                    nc.gpsimd.memset(hmk[c][0:64, 64:128], 0.0)
                    nc.gpsimd.memset(hmk[c][64:128, 0:64], 0.0)
                    nc.gpsimd.memset(hmv[c][0:64, 64:128], 0.0)
                    nc.gpsimd.memset(hmv[c][64:128, 0:64], 0.0)


            # ============ front: projections / gates / conv, interleaved ====
            # ordering tuned so each engine queue receives work in expected
            # readiness order (in-order queues suffer head-of-line blocking)
            proj_ps = {}

            def proj_mms(ct, dlist):
                c0 = ct * 128
                if ct not in proj_ps:
                    proj_ps[ct] = [ps_all.tile([128, 512], f32, tag="ps",
                                               name=f"pj{ct}{th}") for th in range(2)]
                for d in dlist:
                    for th in range(2):
                        nc.tensor.matmul(proj_ps[ct][th][:], wcat_sb[:, d, c0:c0 + 128],
                                         xt16_sb[:, d, th * 512:(th + 1) * 512],
                                         start=(d == 0), stop=(d == 7))

            def proj_copies(ct, R):
                for th in range(2):
                    dst = R[:, 3 + th * 512: 3 + (th + 1) * 512]
                    nc.vector.tensor_copy(dst, proj_ps[ct][th][:])

            def conv_mms(tile_idx, R):
                cps = [ps_all.tile([128, 512], f32, tag="ps", name=f"cv{tile_idx}{th}")
                       for th in range(2)]
                for i in range(KW):
                    for th in range(2):
                        nc.tensor.matmul(cps[th][:],
                                         convd_sb[:, tile_idx * 4 + i, :],
                                         R[:, i + th * 512: i + th * 512 + 512],
                                         start=(i == 0), stop=(i == KW - 1))
                return cps

            # -- kv path first: projection, conv, silu, transposes --
            proj_mms(1, range(0, 8))
            proj_copies(1, Rkv)
            cps_kv = conv_mms(1, Rkv)
            for th in range(2):
                nc.scalar.activation(Skv[:, th * 512:(th + 1) * 512], cps_kv[th][:], AF.Silu)
            nc.sync.dma_start(out=kst[0:64, :], in_=Skv[0:64, :])
            nc.scalar.dma_start(out=kst[64:128, :], in_=Skv[0:64, :])
            for c in range(NCH):
                eng = nc.sync if c % 2 == 0 else nc.scalar
                eng.dma_start(out=kvt[c][:], in_=Skv[:, c * L:(c + 1) * L],
                              transpose=True)

            # -- gate projection + transposes --
            pg = [ps_all.tile([5, 512], f32, tag="ps", name=f"pg{th}") for th in range(2)]
            for d in range(8):
                for th in range(2):
                    nc.tensor.matmul(pg[th][:], wg5_sb[:, d, :],
                                     xt16_sb[:, d, th * 512:(th + 1) * 512],
                                     start=(d == 0), stop=(d == 7))
            for th in range(2):
                nc.scalar.copy(gF[:, th * 512:(th + 1) * 512], pg[th][:])
            ps_gt = ps_all.tile([128, 64], f16, tag="ps", name="psgt")
            for c in range(NCH):
                nc.tensor.transpose(ps_gt[:, c * 8:c * 8 + 5],
                                    gF[:, c * 128:(c + 1) * 128],
                                    iden16_sb[0:5, 0:5])
            nc.vector.tensor_copy(g_tmT[:], ps_gt[:])

            # -- gate chain (Act/DVE) --
            gv = g_tmT[:].rearrange("p (c r) -> p c r", r=8)
            eav = e_a[:].rearrange("p (c h) -> p c h", h=2)
            for h in range(2):
                nc.scalar.activation(eav[:, :, h], gv[:, :, h], AF.Exp,
                                     bias=dtb_sb[:, h:h + 1])
            nc.scalar.activation(sp_a[:], e_a[:], AF.Ln, bias=1.0)
            nc.scalar.activation(era[:], alog_sb[:], AF.Exp)
            ps_bc = ps_all.tile([128, 128], f32, tag="ps", name="psbc")
            nc.tensor.matmul(ps_bc[:, 0:2], ones128h[:], era[:], start=True, stop=True)
            nc.vector.tensor_scalar(negea[:], ps_bc[:, 0:2], -1.0, None, ALU.mult)
            spv = sp_a[:].rearrange("p (c h) -> p c h", h=2)
            gav = g_all[:].rearrange("p (c h) -> p c h", h=2)
            for h in range(2):
                nc.vector.tensor_scalar(gav[:, :, h], spv[:, :, h],
                                        negea[:, h:h + 1], None, ALU.mult)
            nc.scalar.activation(e_ab[:].rearrange("p (c r) -> p c r", r=3),
                                 gv[:, :, 2:5], AF.Exp, scale=-1.0)
            nc.vector.tensor_scalar(d_ab[:], e_ab[:], 1.0, None, ALU.add)
            nc.vector.reciprocal(ab_all[:], d_ab[:])
            nc.scalar.activation(asq[:].rearrange("p (c h) -> p c h", h=2),
                                 ab_all[:].rearrange("p (c r) -> p c r", r=3)[:, :, 0:2],
                                 AF.Square)

            # -- k-norm squares (DVE; Act Ln/Exp deferred past the silus) --
            for c in range(NCH):
                nc.vector.tensor_tensor(sqk_all[:, c * 64:(c + 1) * 64],
                                        kvt[c][:, 0:64], kvt[c][:, 0:64], ALU.mult)

            # -- q path: q-proj + q-conv + silus early; masks after --
            proj_mms(0, range(0, 4))
            psG = ps_all.tile([128, 16], f32, tag="ps", name="psG")
            nc.tensor.matmul(psG[:], um_sb[:], g_all[:], start=True, stop=True)
            nc.vector.tensor_scalar(negG[:], psG[:], -1.0, None, ALU.mult)
            psGr = ps_all.tile([16, 128], f32, tag="ps", name="psGr")
            nc.tensor.matmul(psGr[:], g_all[:], um_sb[:], start=True, stop=True)
            nc.vector.tensor_copy(growall[:], psGr[:])
            nc.vector.tensor_copy(growhi[:], psGr[:])
            nc.vector.tensor_tensor(growres[:], growall[:], growhi[:], ALU.subtract)
            nc.sync.dma_start(out=growrowh[:], in_=growhi[:])
            nc.scalar.dma_start(out=growrowr[:], in_=growres[:])
            nc.vector.tensor_copy(gamlog16[:], psGr[:])
            proj_mms(0, range(4, 8))
            proj_copies(0, Rq)
            cps_q = conv_mms(0, Rq)
            for th in range(2):
                nc.scalar.activation(Sq[:, th * 512:(th + 1) * 512], cps_q[th][:], AF.Silu)

            # k-norm invk (Act Ln/Exp, after the silu block in queue order)
            for c in range(1, NCH, 2):
                nc.vector.tensor_reduce(
                    ssk[:, c - 1:c + 1],
                    sqk_all[:, (c - 1) * 64:(c + 1) * 64]
                    .rearrange("p (c k) -> p c k", c=2), X, ALU.add)
                nc.scalar.activation(lnk[:, c - 1:c + 1], ssk[:, c - 1:c + 1],
                                     AF.Ln, bias=b1em12[:])
                nc.scalar.activation(invk[:, c - 1:c + 1], lnk[:, c - 1:c + 1],
                                     AF.Exp, scale=-0.5)

            # gamma broadcast (log-domain selector matmul, exp)
            for b in range(2):
                psgb = ps_all.tile([128, 512], f32, tag="ps", name=f"psgb{b}")
                for k in range(4):
                    c = b * 4 + k
                    nc.tensor.matmul(psgb[:, k * 128:(k + 1) * 128],
                                     selall_sb[:, c * 128:(c + 1) * 128],
                                     gamlog16[:], start=True, stop=True)
                for k in range(2):
                    nc.scalar.activation(gb_st[:, b * 512 + k * 256: b * 512 + (k + 1) * 256],
                                         psgb[:, k * 256:(k + 1) * 256], AF.Exp)
            nc.vector.tensor_copy(
                glf[:], gb_st[:].rearrange("p (c t) -> p c t", c=NCH)[:, :, L - 1])

            # decay masks
            for b in range(4):
                psGB = ps_all.tile([128, 512], f32, tag="ps", name=f"psGB{b}")
                nc.tensor.matmul(psGB[:], ones128h[:], growrowh[:, b * 512:(b + 1) * 512],
                                 start=True, stop=False)
                nc.tensor.matmul(psGB[:], ones128h[:], growrowr[:, b * 512:(b + 1) * 512],
                                 start=False, stop=False)
                nc.tensor.matmul(psGB[:], up16_sb[:], rhs30k_sb[:],
                                 start=False, stop=True)
                for k in range(4):
                    r = b * 4 + k
                    nc.scalar.activation(gt_all[:, r * 128:(r + 1) * 128],
                                         psGB[:, k * 128:(k + 1) * 128], AF.Exp,
                                         bias=negG[:, r:r + 1])

            # ============ kvtm + kw + state chain (per chunk) ======
            hm_prev = None
            for c in range(NCH):
                km = kvtm[c]
                nc.vector.tensor_scalar(km[:, 0:64], kvt[c][:, 0:64],
                                        invk[:, c:c + 1], None, ALU.mult)
                nc.vector.tensor_scalar(km[:, 64:128], kvt[c][:, 64:128],
                                        ab_all[:, c * 3 + 2:c * 3 + 3], None, ALU.mult)
                nc.vector.tensor_scalar(km[:, 128:192], km[:, 64:128],
                                        invk[:, c:c + 1], None, ALU.mult)
                nc.gpsimd.tensor_scalar(knegp[c][:, 0:64], km[:, 0:64],
                                        invk[:, c:c + 1], -CHEB_DAMP, ALU.mult, ALU.mult)
                nc.gpsimd.tensor_copy(knegp[c][:, 192:256], knegp[c][:, 0:64])
                for h in range(2):
                    nc.vector.tensor_tensor(
                        kw[c][:, h * 64:(h + 1) * 64], km[:, 0:64],
                        gt_all[:, c * 256 + h * 128 + L - 1: c * 256 + h * 128 + L]
                        .broadcast_to([128, 64]), ALU.mult)
                ps_hm = ps_all.tile([128, 128], f32, tag="ps", name=f"pshm{c}")
                nc.tensor.matmul(ps_hm[:], kw[c][:], kvtm[c][:, 0:128],
                                 start=True, stop=(c == 0))
                if c > 0:
                    dg = p_sm.tile([128, 128], f16, tag="diag", name=f"diag{c}")
                    nc.vector.tensor_scalar(dg[:], iden16_sb[:],
                                            glf[:, c:c + 1], None, ALU.mult)
                    nc.tensor.matmul(ps_hm[:], dg[:], hm_prev[:],
                                     start=False, stop=True)
                if c + 1 < NCH:
                    hm = p_kv.tile([128, 128], f16, tag="hm", name=f"hm{c}")
                    nc.vector.tensor_copy(hm[:], ps_hm[:])
                    hm_prev = hm
                    cn = c + 1
                    nc.vector.tensor_scalar(hmk[cn][0:64, 0:64], ps_hm[0:64, 0:64],
                                            -CHEB_DAMP, None, ALU.mult)
                    nc.vector.tensor_scalar(hmk[cn][64:128, 64:128], ps_hm[64:128, 0:64],
                                            -CHEB_DAMP, None, ALU.mult)
                    nc.vector.tensor_copy(hmv[cn][0:64, 0:64], ps_hm[0:64, 64:128])
                    nc.vector.tensor_copy(hmv[cn][64:128, 64:128], ps_hm[64:128, 64:128])

            if dbg and _rep == 0:
                nc.sync.dma_start(out=dSq[:], in_=Sq[:])
                nc.sync.dma_start(out=dSkv[:], in_=Skv[:])
                nc.sync.dma_start(out=dgtm[:], in_=g_tmT[:])
                nc.sync.dma_start(out=dgt[:], in_=gt_all[:])
                nc.sync.dma_start(out=dgb[:], in_=gb_st[:])
                nc.sync.dma_start(out=dinvk[:], in_=invk[:])
                for c in range(NCH):
                    nc.sync.dma_start(out=dkvtm[c], in_=kvtm[c][:])
                    if c > 0:
                        nc.sync.dma_start(out=dhmk[c], in_=hmk[c][:])
                        nc.sync.dma_start(out=dhmv[c], in_=hmv[c][:])

            # stacked q into block-diag quadrants (feeds pass-2 scores)
            qv = qse[:].rearrange("p (c a t) -> p c a t", c=NCH, a=2)
            nc.vector.tensor_copy(
                qv[0:64, :, 0, :],
                Sq[0:64, :].rearrange("p (c t) -> p c t", c=NCH))
            nc.vector.tensor_copy(
                qv[64:128, :, 1, :],
                Sq[64:128, :].rearrange("p (c t) -> p c t", c=NCH))

            # ============ pass 2: operator applications ============
            grp = list(range(NCH))
            xcur = {ci: qse[:, ci * 256:(ci + 1) * 256] for ci in grp}
            asb = {}
            ps_o = {}
            for it in range(3):
                ps_p = {}
                pspb_by = {}
                for ci in grp:
                    if ci % 2 == 0:
                        pspb = ps_all.tile([128, 512], f32, tag="ps", name=f"pspb{ci}")
                    pspb_by[ci] = pspb
                    ps_p[ci] = pspb[:, (ci % 2) * 256:(ci % 2 + 1) * 256]
                    nc.tensor.matmul(ps_p[ci], kst[:, ci * L:(ci + 1) * L],
                                     xcur[ci], start=True, stop=True)
                for ci in grp:
                    if ci % 2 == 1:
                        scr = p_kv.tile([128, 512], f16, tag="scr", name=f"scr{ci}")
                        nc.scalar.copy(scr[:], pspb_by[ci][:])
                        a2 = p_kv.tile([128, 512], f16, tag="a", name=f"asb{ci}")
                        nc.vector.tensor_tensor(
                            a2[:], scr[:],
                            gt_all[:, (ci - 1) * 256:(ci + 1) * 256], ALU.mult)
                        asb[ci - 1] = a2[:, 0:256]
                        asb[ci] = a2[:, 256:512]
                xgs = {}
                for ci in grp:
                    if ci > 0:
                        xs = p_xq.tile([128, 128], f16, tag="xgs", name=f"xgs{ci}")
                        nc.vector.tensor_tensor(
                            xs[0:64, :], xcur[ci][0:64, 0:128],
                            gb_st[0:64, ci * 128:(ci + 1) * 128], ALU.mult)
                        nc.gpsimd.tensor_tensor(
                            xs[64:128, :], xcur[ci][64:128, 128:256],
                            gb_st[64:128, ci * 128:(ci + 1) * 128], ALU.mult)
                        xgs[ci] = xs
                if it < 2:
                    ps_y = {}
                    for ci in grp:
                        if ci % 4 == 0:
                            psyb = ps_oy.tile([128, 512], f32, tag="psy", name=f"psyb{it}{ci}")
                        ps_y[ci] = psyb[:, (ci % 4) * 128:(ci % 4 + 1) * 128]
                        nc.tensor.matmul(ps_y[ci], knegp[ci][:, 0:128], asb[ci][:, 0:128],
                                         start=True, stop=False)
                        nc.tensor.matmul(ps_y[ci], knegp[ci][:, 128:256], asb[ci][:, 128:256],
                                         start=False, stop=(ci == 0))
                        if ci > 0:
                            nc.tensor.matmul(ps_y[ci], hmk[ci][:], xgs[ci][:],
                                             start=False, stop=True)
                        if ci % 4 == 3:
                            p0 = ci - 3
                            scry = p_xq.tile([128, 512], f16, tag="scry", name=f"scry{it}{ci}")
                            nc.scalar.copy(scry[:], psyb[:])
                            for cj in range(p0, p0 + 4):
                                rg = (cj % 4) * 128
                                nc.vector.tensor_tensor(
                                    xceall[0:64, cj * 256:cj * 256 + 128],
                                    scry[0:64, rg:rg + 128],
                                    qse[0:64, cj * 256:cj * 256 + 128], ALU.add)
                                nc.vector.tensor_tensor(
                                    xceall[64:128, cj * 256 + 128:(cj + 1) * 256],
                                    scry[64:128, rg:rg + 128],
                                    qse[64:128, cj * 256 + 128:(cj + 1) * 256], ALU.add)
                                xcur[cj] = xceall[:, cj * 256:(cj + 1) * 256]
                else:
                    for ci in grp:
                        if ci % 4 == 0:
                            psob = ps_oy.tile([128, 512], f32, tag="psy", name=f"psob{ci}")
                        ps_o[ci] = psob[:, (ci % 4) * 128:(ci % 4 + 1) * 128]
                        for h in range(2):
                            nc.tensor.matmul(ps_o[ci][:, h * 64:(h + 1) * 64],
                                             asb[ci][:, h * 128:(h + 1) * 128],
                                             kvtm[ci][:, 128:192],
                                             start=True, stop=(ci == 0))
                            if ci > 0:
                                nc.tensor.matmul(ps_o[ci][:, h * 64:(h + 1) * 64],
                                                 xgs[ci][:],
                                                 hmv[ci][:, h * 64:(h + 1) * 64],
                                                 start=False, stop=True)

            # ---- rmsnorm + alpha gate + out projection, in two halves ----
            out_rr = 0
            for half in range(2):
                hgrp = list(range(half * 4, half * 4 + 4))
                for ci in hgrp:
                    nc.scalar.activation(sq_all[:, ci * 128:(ci + 1) * 128],
                                         ps_o[ci], AF.Square)
                g0 = half * 4
                sl8 = slice(g0 * 2, g0 * 2 + 8)
                nc.vector.tensor_reduce(
                    sso[:, sl8],
                    sq_all[:, g0 * 128:(g0 + 4) * 128]
                    .rearrange("p (r k) -> p r k", k=64), X, ALU.add)
                nc.vector.tensor_tensor(uvar[:, sl8], sso[:, sl8], asq[:, sl8], ALU.mult)
                nc.scalar.activation(lno[:, sl8], uvar[:, sl8], AF.Ln,
                                     bias=bepsk[:], scale=1.0 / HV)
                nc.scalar.activation(sfac[:, sl8], lno[:, sl8], AF.Exp, scale=-0.5)
                nc.vector.tensor_tensor(
                    sfin[:].rearrange("p (c h) -> p c h", h=2)[:, g0:g0 + 4, :],
                    sfac[:].rearrange("p (c h) -> p c h", h=2)[:, g0:g0 + 4, :],
                    ab_all[:].rearrange("p (c r) -> p c r", r=3)[:, g0:g0 + 4, 0:2],
                    ALU.mult)
                for ci in hgrp:
                    o = p_out.tile([128, 128], f16, tag="on", name=f"on{ci}")
                    nc.vector.tensor_tensor(
                        o[:].rearrange("p (h v) -> p h v", h=2), 
                        ps_o[ci][:].rearrange("p (h v) -> p h v", h=2),
                        sfin[:, ci * 2:ci * 2 + 2].unsqueeze(2).broadcast_to([128, 2, 64]),
                        ALU.mult)
                    ps_of = ps_all.tile([128, 128], f16, tag="ps", name=f"psof{ci}")
                    nc.tensor.transpose(ps_of[:], o[:], iden16_sb[:])
                    f = p_out.tile([128, 128], f16, tag="ofm", name=f"ofm{ci}")
                    nc.vector.tensor_copy(f[:], ps_of[:])
                    out_sb = p_out.tile([128, D], f16, tag="outsb")
                    for nh in range(2):
                        ps_out = ps_all.tile([128, 512], f32, tag="ps", name=f"pso2{ci}{nh}")
                        nc.tensor.matmul(ps_out[:], f[:],
                                         wo_sb[:, nh * 512:(nh + 1) * 512],
                                         start=True, stop=True)
                        dst = out_sb[:, nh * 512:(nh + 1) * 512]
                        nc.scalar.copy(dst, ps_out[:])
                    nc.sync.dma_start(out=outp[ci * L:(ci + 1) * L, 0:512],
                                      in_=out_sb[:, 0:512])
                    nc.scalar.dma_start(out=outp[ci * L:(ci + 1) * L, 512:1024],
                                        in_=out_sb[:, 512:1024])

    # Activation-table placement: map Exp/Ln/Square/Copy into the combined
    # natural_log_exp table (hoisted first) so only Silu forces a switch.
    import concourse.bacc as bacc_mod
    from concourse.hw_specs import get_activation_tables as _gat
    orig_tables = _gat(nc.m.arch)
    orig_names = list(orig_tables.keys())
    pref = "natural_log_exp_and_others"
    reordered = {pref: orig_tables[pref],
                 **{k: v for k, v in orig_tables.items() if k != pref}}
    pnames = list(reordered.keys())
    bacc_mod.get_activation_tables = lambda arch: reordered
    try:
        nc.compile()
    finally:
        bacc_mod.get_activation_tables = _gat
    for b in nc.main_func.blocks:
        for i in b.instructions:
            if isinstance(i, mybir.InstLoadActFuncSet):
                i.act_func_set_id = orig_names.index(pnames[i.act_func_set_id])
    return nc


def _prep_core_inputs(c, x, Wq, Wk, Wv, Wconv, Wa, Walpha, Wb, A_log, dt_bias,
                      norm_w, Wo, xT16, iden16, um32, up16, rhs30k, selall):
    f32, f16 = np.float32, np.float16
    h0, h1, hk = 2 * c, 2 * c + 1, c // 2
    wcat = np.hstack([
        Wq[:, h0 * HK:(h0 + 1) * HK], Wq[:, h1 * HK:(h1 + 1) * HK],
        Wk[:, hk * HK:(hk + 1) * HK], Wv[:, hk * HV:(hk + 1) * HV],
    ]).astype(f16)
    wg5 = np.hstack([
        Wa[:, h0:h0 + 1], Wa[:, h1:h1 + 1],
        Walpha[:, h0:h0 + 1], Walpha[:, h1:h1 + 1],
        Wb[:, hk:hk + 1],
    ]).astype(f16)
    qoff, koff, voff = 0, HQ * HK, HQ * HK + HKV * HK
    wcv = np.vstack([
        Wconv[qoff + h0 * HK: qoff + (h0 + 1) * HK],
        Wconv[qoff + h1 * HK: qoff + (h1 + 1) * HK],
        Wconv[koff + hk * HK: koff + (hk + 1) * HK],
        Wconv[voff + hk * HV: voff + (hk + 1) * HV],
    ]).astype(f32)  # [256, 4]: rows = [q(128) | k(64) | v(64)]
    convd = np.zeros((128, 8, 128), f32)
    for tile in range(2):
        ch = wcv[tile * 128:(tile + 1) * 128]   # [128, KW]
        for tap in range(KW):
            np.fill_diagonal(convd[:, tile * 4 + tap, :], ch[:, tap])
    convd = np.ascontiguousarray(convd.reshape(128, 8 * 128)).astype(f16)
    wcvq = np.ascontiguousarray(wcv[0:128]).astype(f32)
    wo_scale = np.tile(np.asarray(norm_w, f32), HQ)
    Wo_s = np.asarray(Wo, f32) * wo_scale[:, None]
    wo = np.ascontiguousarray(
        np.vstack([Wo_s[h0 * HV:(h0 + 1) * HV], Wo_s[h1 * HV:(h1 + 1) * HV]])).astype(f16)
    alog = np.asarray(A_log, f32)[[h0, h1]].reshape(1, 2).copy()
    dtbbc = np.tile(np.asarray(dt_bias, f32)[[h0, h1]].reshape(1, 2), (128, 1)).copy()
    return dict(xT16=xT16, wcat=np.ascontiguousarray(wcat), wg5=wg5,
                convd=convd, wcvq=wcvq, wo=wo, alog=alog, dtbbc=dtbbc,
                um=um32, up16=up16, rhs30k=rhs30k, iden=iden16, selall=selall)


def make_in_maps(x, Wq, Wk, Wv, Wconv, Wa, Walpha, Wb, A_log, dt_bias, norm_w, Wo):
    f32, f16 = np.float32, np.float16
    x2 = np.asarray(x, f32).reshape(T, D)
    xT16 = np.ascontiguousarray(x2.T).astype(f16)
    iden16 = np.eye(128, dtype=f16)
    um32 = np.ascontiguousarray(np.triu(np.ones((128, 128), f32)))
    up16 = np.ascontiguousarray(np.triu(np.ones((128, 128), f16), 1))
    rhs30k = np.ascontiguousarray(
        np.hstack([-30000.0 * np.eye(128)] * 4)).astype(f16)
    selall = np.zeros((16, 8, 128), f16)
    for c in range(8):
        selall[c * 2, c, 0:64] = 1.0
        selall[c * 2 + 1, c, 64:128] = 1.0
    selall = np.ascontiguousarray(selall.reshape(16, 1024))
    args = (x, np.asarray(Wq, f32), np.asarray(Wk, f32), np.asarray(Wv, f32),
            np.asarray(Wconv, f32), np.asarray(Wa, f32), np.asarray(Walpha, f32),
            np.asarray(Wb, f32), A_log, dt_bias, norm_w, Wo)
    return [_prep_core_inputs(c, *args, xT16=xT16, iden16=iden16, um32=um32,
                              up16=up16, rhs30k=rhs30k, selall=selall)
            for c in range(NCORES)]


def get_program(dbg=False, reps=1):
    key = (dbg, reps)
    if key not in _PROG_CACHE:
        _PROG_CACHE[key] = _build_program(dbg, reps)
    return _PROG_CACHE[key]


def kernel(**inputs) -> np.ndarray:
    from concourse.bass_utils import run_bass_kernel_spmd
    nc = get_program(dbg=False)
    in_maps = make_in_maps(**inputs)
    res = run_bass_kernel_spmd(nc, in_maps, list(range(NCORES)))
    out = np.zeros((T, D), np.float32)
    for c in range(NCORES):
        out += res.results[c]["outp"].astype(np.float32)
    return out.reshape(B, T, D)
